# revision 11
# baseline (speedup 1.0000x reference)
"""Trainium2 Bass kernel for nn_DetectionLoss (B=8, A=3, H=W=80, C=80, M=100).

Data-parallel: image b -> core b (8 cores). Each core computes its image's
partial sums [pos_cnt, sum_l0, sum_posf*l1d, -sum_giou*posf, sum_row*posf];
host combines.

v2 design:
- Anchors padded 150->160 per partition (dummy anchors produce ip=0 -> never
  positive); chunk width NC=32 so the one-hot tile free size (NT*NC=3200) is
  XBAR-divisible.
- IoU core + ranking on DVE in [P,NT,NC] bf16; argmax tie-broken via rife
  (reversed-iota) max; exclusive one-hot emitted in [P,NC,NT] layout.
- Matched-payload select on the (otherwise idle) PE: one dma_start_transpose
  per chunk turns the one-hot into [t, n, p]; 32 tiny matmuls against a
  per-target payload table ([thx,tlx,thy,tly,tae,label]) land the matched
  payloads in PSUM in anchor-partition layout. No gather, no DVE select.
- Focal loss: sigmoid/ln sweeps on Scalar; p^2*ln(1-p) row sums and the
  label-column (s_y) select run on GpSimd, per-chunk, hidden under the DVE
  IoU loop. Correction terms from plane-level ACT ops.
- Per-pair 1/S via reciprocal_approx_fast (f32) + bf16 cast.
- Cross-partition reduce via gpsimd.partition_all_reduce.
"""
import numpy as np

import concourse.bass as bass
import concourse.bacc as bacc
import concourse.mybir as mybir
import concourse.tile as tile
from concourse import bass_isa

F32 = mybir.dt.float32
BF16 = mybir.dt.bfloat16
ALU = mybir.AluOpType
ACTF = mybir.ActivationFunctionType
AX = mybir.AxisListType

P = 128          # partitions
NPP = 150        # real anchors per partition
NPA = 160        # padded anchors per partition
N = P * NPP      # 19200 real anchors
NT = 100         # targets
C = 80           # classes
NC = 32          # anchor chunk width (NT*NC % 128 == 0 for XBAR transpose)
NCH = NPA // NC  # 5 chunks
NPAY = 6         # payload slots: thx, tlx, thy, tly, tae, label
B = 8
THIRD = 1.0 / 3.0


def build_kernel(debug_taps=False):
    nc = bacc.Bacc(None, target_bir_lowering=False, debug=False)

    obj_d = nc.dram_tensor("obj", [P, NPP], F32, kind="ExternalInput")
    af_d = nc.dram_tensor("af", [P, 5, NPA], F32, kind="ExternalInput")
    ab_d = nc.dram_tensor("ab", [P, 5, NPA], BF16, kind="ExternalInput")
    cls_d = nc.dram_tensor("cls", [P, NPA * C], F32, kind="ExternalInput")
    teall_d = nc.dram_tensor("teall", [P, NT, NPAY * NC], BF16,
                             kind="ExternalInput")
    rife_d = nc.dram_tensor("rife", [P, NT * NC], BF16, kind="ExternalInput")
    rife2_d = nc.dram_tensor("rife2", [P, NC * NT], BF16, kind="ExternalInput")
    tab_d = nc.dram_tensor("tab", [P, 8], BF16, kind="ExternalInput")
    cif_d = nc.dram_tensor("cif", [P, C], BF16, kind="ExternalInput")
    out_d = nc.dram_tensor("out", [1, 8], F32, kind="ExternalOutput")
    if debug_taps:
        doh_d = nc.dram_tensor("doh", [P, NC * P], BF16,
                               kind="ExternalOutput")
        dohT_d = nc.dram_tensor("dohT", [P, NC * P], BF16,
                                kind="ExternalOutput")
        dpayl_d = nc.dram_tensor("dpayl", [P, 6, NPA], F32,
                                 kind="ExternalOutput")

    with nc.allow_low_precision("bf16 iou/focal phases are tolerance-analyzed"), \
         tile.TileContext(nc) as tc:
        with tc.tile_pool(name="const", bufs=1) as cpool, \
             tc.tile_pool(name="planes", bufs=1) as ppool, \
             tc.tile_pool(name="iou", bufs=1) as ipool, \
             tc.tile_pool(name="iouf", bufs=1) as fpool32, \
             tc.tile_pool(name="oh", bufs=1) as opool, \
             tc.tile_pool(name="foc", bufs=2) as fpool, \
             tc.tile_pool(name="focs", bufs=1) as fspool, \
             tc.tile_pool(name="psum", bufs=2, space="PSUM") as qpool:

            def plane(tag, dt=F32):
                return ppool.tile([P, NPA], dt, tag=tag, name=tag)

            # ---------- resident loads ----------
            ab_t = cpool.tile([P, 5, NPA], BF16)
            nc.sync.dma_start(ab_t[:], ab_d[:])
            teall_t = cpool.tile([P, NT, NPAY, NC], BF16)
            nc.sync.dma_start(
                teall_t[:].rearrange("p t j n -> p t (j n)"), teall_d[:])
            rife_t = cpool.tile([P, NT, NC], BF16)
            nc.sync.dma_start(
                rife_t[:].rearrange("p t n -> p (t n)"), rife_d[:])
            rife2_t = cpool.tile([P, NC, NT], BF16)
            nc.sync.dma_start(
                rife2_t[:].rearrange("p n t -> p (n t)"), rife2_d[:])
            tab_t = cpool.tile([P, 8], BF16)
            nc.sync.dma_start(tab_t[:], tab_d[:])
            cif_t = cpool.tile([P, C], BF16)
            nc.sync.dma_start(cif_t[:], cif_d[:])
            af_t = cpool.tile([P, 5, NPA], F32)
            nc.sync.dma_start(af_t[:], af_d[:])
            obj_t = cpool.tile([P, NPP], F32)
            nc.sync.dma_start(obj_t[:], obj_d[:])

            part_t = ppool.tile([P, 8], F32)
            nc.vector.memset(part_t[:, 5:8], 0.0)

            mxf_t = plane("mxf")                    # max g per anchor (f32)
            payl_t = cpool.tile([P, 5, NPA], F32)   # matched payload planes
            ylb_t = plane("ylb", BF16)              # matched label (bf16)
            rs0_t = plane("rs0")                    # sum_c p^2 ln(1-p)
            sy_t = plane("sy")                      # logit at label column
            posf_t = plane("posf")

            cls3 = cls_d[:].rearrange("p (n c) -> p n c", c=C)

            def tree1(scratch, src, w, op):
                first = True
                while w > 1:
                    h = w // 2
                    s = src if first else scratch
                    nc.vector.tensor_tensor(scratch[:, 0:h], s[:, 0:h],
                                            s[:, h:2 * h], op)
                    if w % 2:
                        nc.vector.tensor_tensor(scratch[:, 0:1],
                                                scratch[:, 0:1],
                                                s[:, w - 1:w], op)
                    first = False
                    w = h
                return scratch

            def tree_last(scratch, src, w, op):
                first = True
                while w > 1:
                    h = w // 2
                    s = src if first else scratch
                    nc.vector.tensor_tensor(scratch[:, :, 0:h], s[:, :, 0:h],
                                            s[:, :, h:2 * h], op)
                    if w % 2:
                        nc.vector.tensor_tensor(scratch[:, :, 0:1],
                                                scratch[:, :, 0:1],
                                                s[:, :, w - 1:w], op)
                    first = False
                    w = h
                return scratch

            def aexp(j, c0):
                return ab_t[:, j, c0:c0 + NC].unsqueeze(1) \
                    .broadcast_to([P, NT, NC])

            cie = cif_t[:].unsqueeze(1).broadcast_to([P, NC, C])

            # one-hot staging: [P, NC, 128] (t padded to 128), zero once
            oh2 = cpool.tile([P, NC, P], BF16, tag="oh2c", name="oh2c")
            nc.vector.memset(oh2[:], 0.0)
            ohT = cpool.tile([P, NC, P], BF16, tag="ohTc", name="ohTc")

            # ---------- main chunk loop ----------
            for ci in range(NCH):
                c0 = ci * NC
                # cls chunk DMA; sigmoid + ln(1-p) on scalar
                sc = fpool.tile([P, NC, C], F32, tag="sc", name="sc", bufs=3)
                nc.sync.dma_start(sc[:], cls3[:, c0:c0 + NC, :])
                pb = fpool.tile([P, NC, C], BF16, tag="pb", name="pb")
                nc.scalar.activation(pb[:], sc[:], ACTF.Sigmoid)
                lnp = fpool.tile([P, NC, C], BF16, tag="lnp", name="lnp")
                nc.scalar.activation(lnp[:], pb[:], ACTF.Ln, bias=1.0,
                                     scale=-1.0)

                # ---- DVE: IoU core + ranking ----
                ta = ipool.tile([P, NT, NC], BF16, tag="ta", name="ta")
                tb = ipool.tile([P, NT, NC], BF16, tag="tb", name="tb")
                tc2 = ipool.tile([P, NT, NC], BF16, tag="tc", name="tc")
                td = ipool.tile([P, NT, NC], BF16, tag="td", name="td")
                s32 = fpool32.tile([P, NT, NC], F32, tag="s32", name="s32")
                rsb = fpool32.tile([P, NT, NC], BF16, tag="rsb", name="rsb")

                nc.vector.tensor_tensor(ta[:], aexp(0, c0),
                                        teall_t[:, :, 0, :], ALU.min)   # hx
                nc.vector.tensor_tensor(tb[:], aexp(1, c0),
                                        teall_t[:, :, 1, :], ALU.max)   # lx
                nc.vector.tensor_sub(ta[:], ta[:], tb[:])               # wx
                nc.vector.tensor_single_scalar(td[:], ta[:], 0.0, ALU.max)
                nc.vector.tensor_tensor(ta[:], aexp(2, c0),
                                        teall_t[:, :, 2, :], ALU.min)   # hy
                nc.vector.tensor_tensor(tb[:], aexp(3, c0),
                                        teall_t[:, :, 3, :], ALU.max)   # ly
                nc.vector.tensor_sub(ta[:], ta[:], tb[:])               # wy
                nc.vector.tensor_single_scalar(tb[:], ta[:], 0.0, ALU.max)
                nc.vector.tensor_mul(td[:], td[:], tb[:])               # ip
                nc.vector.tensor_tensor(s32[:], teall_t[:, :, 4, :],
                                        aexp(4, c0), ALU.add)           # S
                nc.vector.reciprocal_approx_fast(s32[:], s32[:])        # 1/S
                nc.vector.tensor_copy(rsb[:], s32[:])                   # bf16
                nc.vector.tensor_mul(tc2[:], td[:], rsb[:])             # g
                mx = tree1(tb, tc2, NT, ALU.max)
                mxe = mx[:, 0:1, :].broadcast_to([P, NT, NC])
                nc.vector.tensor_tensor(ta[:], tc2[:], mxe, ALU.is_equal)
                nc.vector.tensor_mul(ta[:], ta[:], rife_t[:])           # rsel
                rmx = tree1(td, ta, NT, ALU.max)
                # exclusive one-hot in [P, NC, NT] layout (t innermost)
                rme2m = opool.tile([P, NC, NT], BF16, tag="rme2m",
                                   name="rme2m")
                nc.scalar.copy(rme2m[:], rmx[:, 0, :].unsqueeze(2)
                               .broadcast_to([P, NC, NT]))
                nc.vector.tensor_tensor(oh2[:, :, 0:NT], rife2_t[:],
                                        rme2m[:], ALU.is_equal)
                nc.scalar.copy(mxf_t[:, c0:c0 + NC], mx[:, 0:1, :].squeeze(1))

                # ---- PE: payload select via transpose + matmul ----
                for n in range(NC):
                    nc.sync.dma_start_transpose(ohT[:, n, :], oh2[:, n, :])
                psum = qpool.tile([P, NC, NPAY], F32, tag="ps", name="ps")
                for n in range(NC):
                    nc.tensor.matmul(psum[:, n, :], ohT[0:NT, n, :],
                                     tab_t[0:NT, 0:NPAY])
                for j in range(5):
                    nc.scalar.copy(payl_t[:, j, c0:c0 + NC], psum[:, :, j])
                nc.scalar.copy(ylb_t[:, c0:c0 + NC], psum[:, :, 5])
                if debug_taps and ci == 0:
                    nc.sync.dma_start(doh_d[:],
                                      oh2[:].rearrange("p n t -> p (n t)"))
                    nc.sync.dma_start(
                        dohT_d[:],
                        ohT[:].rearrange("p n q -> p (n q)"))

                # ---- DVE: focal row sums + label-column select ----
                scb = fspool.tile([P, NC, C], BF16, tag="scb", name="scb")
                nc.scalar.copy(scb[:], sc[:])
                p2 = fspool.tile([P, NC, C], BF16, tag="p2", name="p2")
                nc.vector.tensor_mul(p2[:], pb[:], pb[:])
                nc.vector.tensor_mul(p2[:], p2[:], lnp[:])
                tree_last(p2, p2, C, ALU.add)
                nc.scalar.copy(rs0_t[:, c0:c0 + NC], p2[:, :, 0:1].squeeze(2))
                syp = fspool.tile([P, NC, C], BF16, tag="syp", name="syp")
                yle = ylb_t[:, c0:c0 + NC].unsqueeze(2) \
                    .broadcast_to([P, NC, C])
                nc.vector.tensor_tensor(syp[:], cie, yle, ALU.is_equal)
                nc.vector.tensor_mul(syp[:], syp[:], scb[:])
                tree_last(syp, syp, C, ALU.add)
                nc.scalar.copy(sy_t[:, c0:c0 + NC], syp[:, :, 0:1].squeeze(2))

            # ---------- objectness BCE ----------
            l0_t = ppool.tile([P, NPP], F32, tag="l0", name="l0")
            nc.scalar.activation(l0_t[:], obj_t[:], ACTF.Ln, bias=1.0,
                                 scale=-1.0)
            l1_t = ppool.tile([P, NPP], F32, tag="l1", name="l1")
            nc.scalar.activation(l1_t[:], obj_t[:], ACTF.Ln)
            nc.vector.tensor_single_scalar(l1_t[:], l1_t[:], -100.0, ALU.max)
            nc.vector.tensor_reduce(part_t[:, 1:2], l0_t[:], AX.X, ALU.add)
            nc.vector.tensor_sub(l1_t[:], l1_t[:], l0_t[:])   # logit diff

            # ---------- pos mask + masked sums ----------
            nc.vector.tensor_single_scalar(posf_t[:], mxf_t[:], THIRD,
                                           ALU.is_ge)
            nc.vector.tensor_reduce(part_t[:, 0:1], posf_t[:], AX.X, ALU.add)
            nc.vector.tensor_mul(l1_t[:], l1_t[:], posf_t[:, 0:NPP])
            nc.vector.tensor_reduce(part_t[:, 2:3], l1_t[:], AX.X, ALU.add)

            # ---------- focal correction planes ----------
            py_t = plane("py")
            nc.scalar.activation(py_t[:], sy_t[:], ACTF.Sigmoid)
            lnpy_t = plane("lnpy")
            nc.scalar.activation(lnpy_t[:], py_t[:], ACTF.Ln)      # = -spny
            ly_t = plane("ly")
            nc.scalar.activation(ly_t[:], py_t[:], ACTF.Ln, bias=1.0,
                                 scale=-1.0)                       # ln(1-py)
            qy_t = plane("qy")
            nc.vector.tensor_scalar(qy_t[:], py_t[:], -1.0, 1.0, ALU.mult,
                                    ALU.add)
            nc.vector.tensor_mul(qy_t[:], qy_t[:], qy_t[:])
            g1_t = plane("g1")
            nc.vector.scalar_tensor_tensor(g1_t[:], lnpy_t[:], -0.25, qy_t[:],
                                           ALU.mult, ALU.mult)     # g1y
            py2_t = plane("py2")
            nc.vector.tensor_mul(py2_t[:], py_t[:], py_t[:])
            g0_t = plane("g0")
            nc.vector.scalar_tensor_tensor(g0_t[:], py2_t[:], -0.75, ly_t[:],
                                           ALU.mult, ALU.mult)     # g0y
            nc.vector.tensor_sub(g1_t[:], g1_t[:], g0_t[:])        # corr
            row_t = plane("row")
            nc.vector.scalar_tensor_tensor(row_t[:], rs0_t[:], -0.75, g1_t[:],
                                           ALU.mult, ALU.add)
            nc.vector.tensor_mul(row_t[:], row_t[:], posf_t[:])
            nc.vector.tensor_reduce(part_t[:, 4:5], row_t[:], AX.X, ALU.add)

            # ---------- GIoU planes (f32) ----------
            thxM = payl_t[:, 0, :]
            tlxM = payl_t[:, 1, :]
            thyM = payl_t[:, 2, :]
            tlyM = payl_t[:, 3, :]
            taeM = payl_t[:, 4, :]
            sM_t = plane("sM")
            nc.vector.tensor_tensor(sM_t[:], af_t[:, 4, :], taeM, ALU.add)
            ipM_t = plane("ipM")
            nc.vector.tensor_mul(ipM_t[:], mxf_t[:], sM_t[:])
            un_t = plane("un")
            nc.vector.tensor_sub(un_t[:], sM_t[:], ipM_t[:])   # union + 1e-6
            ru_t = plane("ru")
            nc.vector.reciprocal_approx_fast(ru_t[:], un_t[:])
            iouM_t = plane("iouM")
            nc.vector.tensor_mul(iouM_t[:], ipM_t[:], ru_t[:])
            ex_t = plane("ex")
            sc2_t = plane("sc2")
            nc.vector.tensor_tensor(ex_t[:], af_t[:, 0, :], thxM, ALU.max)
            nc.vector.tensor_tensor(sc2_t[:], af_t[:, 1, :], tlxM, ALU.min)
            nc.vector.tensor_sub(ex_t[:], ex_t[:], sc2_t[:])
            ey_t = plane("ey")
            nc.vector.tensor_tensor(ey_t[:], af_t[:, 2, :], thyM, ALU.max)
            nc.vector.tensor_tensor(sc2_t[:], af_t[:, 3, :], tlyM, ALU.min)
            nc.vector.tensor_sub(ey_t[:], ey_t[:], sc2_t[:])
            nc.vector.tensor_mul(ex_t[:], ex_t[:], ey_t[:])    # enclose
            nc.vector.tensor_scalar_add(ex_t[:], ex_t[:], 1e-6)
            nc.vector.tensor_sub(ey_t[:], ex_t[:], un_t[:])    # encl - union
            nc.vector.reciprocal_approx_fast(ex_t[:], ex_t[:])
            nc.vector.tensor_mul(ey_t[:], ey_t[:], ex_t[:])
            nc.vector.tensor_sub(iouM_t[:], iouM_t[:], ey_t[:])  # giou
            nc.vector.tensor_mul(iouM_t[:], iouM_t[:], posf_t[:])
            nc.vector.tensor_scalar(iouM_t[:], iouM_t[:], -1.0, 0.0,
                                    ALU.mult, ALU.add)
            nc.vector.tensor_reduce(part_t[:, 3:4], iouM_t[:], AX.X, ALU.add)

            # ---------- cross-partition reduce + final scalars ----------
            red_t = ppool.tile([P, 8], F32)
            nc.gpsimd.partition_all_reduce(red_t[:], part_t[:], P,
                                           bass_isa.ReduceOp.add)
            r0 = red_t[0:1, :]
            out_t = ppool.tile([1, 8], F32)
            nc.vector.memset(out_t[:], 0.0)
            s1 = ppool.tile([1, 1], F32, tag="s1", name="s1")
            nc.vector.tensor_add(s1[:], r0[:, 1:2], r0[:, 2:3])
            c96 = ppool.tile([1, 1], F32, tag="c96", name="c96")
            nc.vector.memset(c96[:], float(N) * 0.5)
            s2 = ppool.tile([1, 1], F32, tag="s2", name="s2")
            nc.vector.scalar_tensor_tensor(s2[:], r0[:, 0:1], 0.5, c96[:],
                                           ALU.mult, ALU.add)
            nc.vector.scalar_tensor_tensor(out_t[:, 0:1], s1[:], -1.0, s2[:],
                                           ALU.mult, ALU.mult)
            nc.vector.tensor_add(out_t[:, 1:2], r0[:, 0:1], r0[:, 3:4])
            s3 = ppool.tile([1, 1], F32, tag="s3", name="s3")
            nc.vector.tensor_scalar(s3[:], r0[:, 0:1], float(C), 1.0,
                                    ALU.mult, ALU.max)
            nc.vector.reciprocal(s3[:], s3[:])
            nc.vector.tensor_mul(out_t[:, 2:3], r0[:, 4:5], s3[:])
            nc.vector.tensor_copy(out_t[:, 3:4], r0[:, 0:1])
            if debug_taps:
                dp = cpool.tile([P, 6, NPA], F32, tag="dp", name="dp")
                nc.scalar.copy(dp[:, 0:5, :], payl_t[:])
                nc.scalar.copy(dp[:, 5, :], ylb_t[:])
                nc.sync.dma_start(dpayl_d[:], dp[:])
            nc.sync.dma_start(out_d[:], out_t[:])

    nc.compile()
    return nc


def prep_core_inputs(objectness, boxes, class_scores, target_boxes,
                     target_labels):
    """Split full inputs into 8 per-core input maps (host-side precompute)."""
    import ml_dtypes
    bf16 = ml_dtypes.bfloat16
    objf = np.ascontiguousarray(objectness, dtype=np.float32).reshape(B, N)
    boxf = np.ascontiguousarray(boxes, dtype=np.float32).reshape(B, N, 4)
    clsf = np.ascontiguousarray(class_scores, dtype=np.float32).reshape(B, N, C)
    tbs = np.asarray(target_boxes, dtype=np.float32)
    tls = np.asarray(target_labels)

    rife1 = np.repeat((199.0 - np.arange(NT, dtype=np.float32))[:, None],
                      NC, axis=1).reshape(NT * NC)
    rife = np.broadcast_to(rife1[None, :], (P, NT * NC)).astype(bf16)
    rife21 = np.repeat((199.0 - np.arange(NT, dtype=np.float32))[None, :],
                       NC, axis=0).reshape(NC * NT)
    rife2 = np.broadcast_to(rife21[None, :], (P, NC * NT)).astype(bf16)
    cif = np.broadcast_to(np.arange(C, dtype=np.float32)[None, :],
                          (P, C)).astype(bf16)

    in_maps = []
    for b in range(B):
        obj = objf[b].reshape(P, NPP)
        bx = boxf[b].reshape(P, NPP, 4)
        cx, cy, w, h = bx[..., 0], bx[..., 1], bx[..., 2], bx[..., 3]
        af = np.zeros((P, 5, NPA), dtype=np.float32)
        af[:, 0, :NPP] = cx + 0.5 * w
        af[:, 1, :NPP] = cx - 0.5 * w
        af[:, 2, :NPP] = cy + 0.5 * h
        af[:, 3, :NPP] = cy - 0.5 * h
        af[:, 4, :NPP] = w * h
        af[:, 0, NPP:] = -20.0   # dummy anchors: zero intersection
        af[:, 1, NPP:] = -19.0
        af[:, 2, NPP:] = -20.0
        af[:, 3, NPP:] = -19.0
        af[:, 4, NPP:] = 1.0
        ab = af.astype(bf16)
        cls = np.zeros((P, NPA, C), dtype=np.float32)
        cls[:, :NPP, :] = clsf[b].reshape(P, NPP, C)
        cls = cls.reshape(P, NPA * C)
        tb = tbs[b]
        pay = np.stack([tb[:, 0] + 0.5 * tb[:, 2],
                        tb[:, 0] - 0.5 * tb[:, 2],
                        tb[:, 1] + 0.5 * tb[:, 3],
                        tb[:, 1] - 0.5 * tb[:, 3],
                        tb[:, 2] * tb[:, 3] + 1e-6,
                        tls[b].astype(np.float32)], axis=1)  # [NT, 6]
        te1 = np.repeat(pay[:, :, None], NC, axis=2).reshape(NT, NPAY * NC)
        teall = np.broadcast_to(te1[None, :, :],
                                (P, NT, NPAY * NC)).astype(bf16)
        tab = np.zeros((P, 8), dtype=np.float32)
        tab[:NT, :NPAY] = pay
        in_maps.append({"obj": obj, "af": af, "ab": np.ascontiguousarray(ab),
                        "cls": cls,
                        "teall": np.ascontiguousarray(teall),
                        "rife": np.ascontiguousarray(rife),
                        "rife2": np.ascontiguousarray(rife2),
                        "tab": tab.astype(bf16),
                        "cif": np.ascontiguousarray(cif)})
    return in_maps


def combine_outputs(outs):
    """outs: list of 8 per-core [1,8] arrays -> scalar loss."""
    o = np.stack([np.asarray(x).reshape(8) for x in outs])  # [8, 8]
    obj_terms, bb_sums, cl_sums, pcs = o[:, 0], o[:, 1], o[:, 2], o[:, 3]
    num_pos = max(float(pcs.sum()), 1.0)
    loss = (np.float32(obj_terms.sum()) / np.float32(B)
            + np.float32(5.0) * np.float32(bb_sums.sum()) / np.float32(num_pos)
            + np.float32(cl_sums.sum()) / np.float32(B))
    return np.float32(loss)


_NC_CACHE = {}


def kernel(objectness, boxes, class_scores, target_boxes, target_labels):
    from concourse.bass_utils import run_bass_kernel_spmd
    if "nc" not in _NC_CACHE:
        _NC_CACHE["nc"] = build_kernel()
    nc = _NC_CACHE["nc"]
    in_maps = prep_core_inputs(objectness, boxes, class_scores,
                               target_boxes, target_labels)
    res = run_bass_kernel_spmd(nc, in_maps, core_ids=list(range(B)))
    outs = [res.results[b]["out"] for b in range(B)]
    return combine_outputs(outs)


# revision 13
# speedup vs baseline: 1.6157x; 1.6157x over previous
"""Trainium2 Bass kernel for nn_DetectionLoss (B=8, A=3, H=W=80, C=80, M=100).

Data-parallel: image b -> core b (8 cores). Each core computes its image's
partial sums [pos_cnt, sum_l0, sum_posf*l1d, -sum_giou*posf, sum_row*posf];
host combines.

v2 design:
- Anchors padded 150->160 per partition (dummy anchors produce ip=0 -> never
  positive); chunk width NC=32 so the one-hot tile free size (NT*NC=3200) is
  XBAR-divisible.
- IoU core + ranking on DVE in [P,NT,NC] bf16; argmax tie-broken via rife
  (reversed-iota) max; exclusive one-hot emitted in [P,NC,NT] layout.
- Matched-payload select on the (otherwise idle) PE: one dma_start_transpose
  per chunk turns the one-hot into [t, n, p]; 32 tiny matmuls against a
  per-target payload table ([thx,tlx,thy,tly,tae,label]) land the matched
  payloads in PSUM in anchor-partition layout. No gather, no DVE select.
- Focal loss: sigmoid/ln sweeps on Scalar; p^2*ln(1-p) row sums and the
  label-column (s_y) select run on GpSimd, per-chunk, hidden under the DVE
  IoU loop. Correction terms from plane-level ACT ops.
- Per-pair 1/S via reciprocal_approx_fast (f32) + bf16 cast.
- Cross-partition reduce via gpsimd.partition_all_reduce.
"""
import numpy as np

import concourse.bass as bass
import concourse.bacc as bacc
import concourse.mybir as mybir
import concourse.tile as tile
from concourse import bass_isa

F32 = mybir.dt.float32
BF16 = mybir.dt.bfloat16
ALU = mybir.AluOpType
ACTF = mybir.ActivationFunctionType
AX = mybir.AxisListType

P = 128          # partitions
NPP = 150        # real anchors per partition
NPA = 160        # padded anchors per partition
N = P * NPP      # 19200 real anchors
NT = 100         # targets
C = 80           # classes
NC = 32          # anchor chunk width (NT*NC % 128 == 0 for XBAR transpose)
NCH = NPA // NC  # 5 chunks
NPAY = 6         # payload slots: thx, tlx, thy, tly, tae, label
B = 8
THIRD = 1.0 / 3.0


def build_kernel(debug_taps=False):
    nc = bacc.Bacc(None, target_bir_lowering=False, debug=False)

    obj_d = nc.dram_tensor("obj", [P, NPP], F32, kind="ExternalInput")
    af_d = nc.dram_tensor("af", [P, 5, NPA], F32, kind="ExternalInput")
    ab_d = nc.dram_tensor("ab", [P, 5, NPA], BF16, kind="ExternalInput")
    cls_d = nc.dram_tensor("cls", [P, NPA * C], F32, kind="ExternalInput")
    teall_d = nc.dram_tensor("teall", [P, NT, NPAY * NC], BF16,
                             kind="ExternalInput")
    rife_d = nc.dram_tensor("rife", [P, NT * NC], BF16, kind="ExternalInput")
    rife2_d = nc.dram_tensor("rife2", [P, NC * NT], BF16, kind="ExternalInput")
    tab_d = nc.dram_tensor("tab", [P, 8], BF16, kind="ExternalInput")
    cif_d = nc.dram_tensor("cif", [P, C], BF16, kind="ExternalInput")
    ident_d = nc.dram_tensor("ident", [P, P], BF16, kind="ExternalInput")
    out_d = nc.dram_tensor("out", [1, 8], F32, kind="ExternalOutput")
    if debug_taps:
        doh_d = nc.dram_tensor("doh", [P, NC * P], BF16,
                               kind="ExternalOutput")
        dohT_d = nc.dram_tensor("dohT", [P, NC * P], BF16,
                                kind="ExternalOutput")
        dpayl_d = nc.dram_tensor("dpayl", [P, 6, NPA], F32,
                                 kind="ExternalOutput")

    with nc.allow_low_precision("bf16 iou/focal phases are tolerance-analyzed"), \
         tile.TileContext(nc) as tc:
        with tc.tile_pool(name="const", bufs=1) as cpool, \
             tc.tile_pool(name="planes", bufs=1) as ppool, \
             tc.tile_pool(name="iou", bufs=1) as ipool, \
             tc.tile_pool(name="iouf", bufs=1) as fpool32, \
             tc.tile_pool(name="oh", bufs=2) as opool, \
             tc.tile_pool(name="foc", bufs=2) as fpool, \
             tc.tile_pool(name="focs", bufs=1) as fspool, \
             tc.tile_pool(name="psum", bufs=2, space="PSUM") as qpool:

            def plane(tag, dt=F32):
                return ppool.tile([P, NPA], dt, tag=tag, name=tag)

            # ---------- resident loads ----------
            ab_t = cpool.tile([P, 5, NPA], BF16)
            nc.sync.dma_start(ab_t[:], ab_d[:])
            teall_t = cpool.tile([P, NT, NPAY, NC], BF16)
            nc.sync.dma_start(
                teall_t[:].rearrange("p t j n -> p t (j n)"), teall_d[:])
            rife_t = cpool.tile([P, NT, NC], BF16)
            nc.sync.dma_start(
                rife_t[:].rearrange("p t n -> p (t n)"), rife_d[:])
            rife2_t = cpool.tile([P, NC, NT], BF16)
            nc.sync.dma_start(
                rife2_t[:].rearrange("p n t -> p (n t)"), rife2_d[:])
            tab_t = cpool.tile([P, 8], BF16)
            nc.sync.dma_start(tab_t[:], tab_d[:])
            cif_t = cpool.tile([P, C], BF16)
            nc.sync.dma_start(cif_t[:], cif_d[:])
            ident_t = cpool.tile([P, P], BF16)
            nc.sync.dma_start(ident_t[:], ident_d[:])
            af_t = cpool.tile([P, 5, NPA], F32)
            nc.sync.dma_start(af_t[:], af_d[:])
            obj_t = cpool.tile([P, NPP], F32)
            nc.sync.dma_start(obj_t[:], obj_d[:])

            part_t = ppool.tile([P, 8], F32)
            nc.vector.memset(part_t[:, 5:8], 0.0)

            mxf_t = plane("mxf")                    # max g per anchor (f32)
            payl_t = cpool.tile([P, 5, NPA], F32)   # matched payload planes
            ylb_t = plane("ylb", BF16)              # matched label (bf16)
            rs0_t = plane("rs0")                    # sum_c p^2 ln(1-p)
            sy_t = plane("sy")                      # logit at label column
            posf_t = plane("posf")

            cls3 = cls_d[:].rearrange("p (n c) -> p n c", c=C)

            def tree1(scratch, src, w, op):
                first = True
                while w > 1:
                    h = w // 2
                    s = src if first else scratch
                    nc.vector.tensor_tensor(scratch[:, 0:h], s[:, 0:h],
                                            s[:, h:2 * h], op)
                    if w % 2:
                        nc.vector.tensor_tensor(scratch[:, 0:1],
                                                scratch[:, 0:1],
                                                s[:, w - 1:w], op)
                    first = False
                    w = h
                return scratch

            def tree_last(scratch, src, w, op):
                first = True
                while w > 1:
                    h = w // 2
                    s = src if first else scratch
                    nc.vector.tensor_tensor(scratch[:, :, 0:h], s[:, :, 0:h],
                                            s[:, :, h:2 * h], op)
                    if w % 2:
                        nc.vector.tensor_tensor(scratch[:, :, 0:1],
                                                scratch[:, :, 0:1],
                                                s[:, :, w - 1:w], op)
                    first = False
                    w = h
                return scratch

            def aexp(j, c0):
                return ab_t[:, j, c0:c0 + NC].unsqueeze(1) \
                    .broadcast_to([P, NT, NC])

            cie = cif_t[:].unsqueeze(1).broadcast_to([P, NC, C])


            # ---------- main chunk loop ----------
            for ci in range(NCH):
                c0 = ci * NC
                # cls chunk DMA; sigmoid + ln(1-p) on scalar
                sc = fpool.tile([P, NC, C], F32, tag="sc", name="sc", bufs=2)
                nc.sync.dma_start(sc[:], cls3[:, c0:c0 + NC, :])
                pb = fpool.tile([P, NC, C], BF16, tag="pb", name="pb")
                nc.scalar.activation(pb[:], sc[:], ACTF.Sigmoid)
                lnp = fpool.tile([P, NC, C], BF16, tag="lnp", name="lnp")
                nc.scalar.activation(lnp[:], pb[:], ACTF.Ln, bias=1.0,
                                     scale=-1.0)

                # ---- DVE: IoU core + ranking ----
                ta = ipool.tile([P, NT, NC], BF16, tag="ta", name="ta")
                tb = ipool.tile([P, NT, NC], BF16, tag="tb", name="tb")
                tc2 = ipool.tile([P, NT, NC], BF16, tag="tc", name="tc")
                td = ipool.tile([P, NT, NC], BF16, tag="td", name="td")
                s32 = fpool32.tile([P, NT, NC], F32, tag="s32", name="s32")
                rsb = fpool32.tile([P, NT, NC], BF16, tag="rsb", name="rsb")

                nc.vector.tensor_tensor(ta[:], aexp(0, c0),
                                        teall_t[:, :, 0, :], ALU.min)   # hx
                nc.vector.tensor_tensor(tb[:], aexp(1, c0),
                                        teall_t[:, :, 1, :], ALU.max)   # lx
                nc.vector.tensor_sub(ta[:], ta[:], tb[:])               # wx
                nc.vector.tensor_single_scalar(td[:], ta[:], 0.0, ALU.max)
                nc.vector.tensor_tensor(ta[:], aexp(2, c0),
                                        teall_t[:, :, 2, :], ALU.min)   # hy
                nc.vector.tensor_tensor(tb[:], aexp(3, c0),
                                        teall_t[:, :, 3, :], ALU.max)   # ly
                nc.vector.tensor_sub(ta[:], ta[:], tb[:])               # wy
                nc.vector.tensor_single_scalar(tb[:], ta[:], 0.0, ALU.max)
                nc.vector.tensor_mul(td[:], td[:], tb[:])               # ip
                nc.vector.tensor_tensor(s32[:], teall_t[:, :, 4, :],
                                        aexp(4, c0), ALU.add)           # S
                nc.vector.reciprocal_approx_fast(s32[:], s32[:])        # 1/S
                nc.vector.tensor_copy(rsb[:], s32[:])                   # bf16
                nc.vector.tensor_mul(tc2[:], td[:], rsb[:])             # g
                mx = tree1(tb, tc2, NT, ALU.max)
                mxe = mx[:, 0:1, :].broadcast_to([P, NT, NC])
                nc.vector.tensor_tensor(ta[:], tc2[:], mxe, ALU.is_equal)
                nc.vector.tensor_mul(ta[:], ta[:], rife_t[:])           # rsel
                rmx = tree1(td, ta, NT, ALU.max)
                # exclusive one-hot in [P, NC, NT] layout (t innermost)
                rme2m = fspool.tile([P, NC, NT], BF16, tag="rme2m",
                                    name="rme2m")
                nc.scalar.copy(rme2m[:], rmx[:, 0, :].unsqueeze(2)
                               .broadcast_to([P, NC, NT]))
                oh2 = opool.tile([P, NC, NT], BF16, tag="oh2", name="oh2")
                nc.vector.tensor_tensor(oh2[:], rife2_t[:], rme2m[:],
                                        ALU.is_equal)
                nc.scalar.copy(mxf_t[:, c0:c0 + NC], mx[:, 0:1, :].squeeze(1))

                # ---- PE: payload select via transpose + matmul ----
                psT = qpool.tile([P, NC, P], BF16, tag="psT", name="psT",
                                 bufs=1)
                for n in range(NC):
                    nc.tensor.transpose(psT[0:NT, n, :], oh2[:, n, :],
                                        ident_t[:])
                ohTs = opool.tile([P, NC, P], BF16, tag="ohTs",
                                  name="ohTs", bufs=1)
                nc.scalar.copy(ohTs[0:NT, :, :], psT[0:NT, :, :])
                psum = qpool.tile([P, NC, NPAY], F32, tag="ps", name="ps")
                for n in range(NC):
                    nc.tensor.matmul(psum[:, n, :], ohTs[0:NT, n, :],
                                     tab_t[0:NT, 0:NPAY])
                for j in range(5):
                    nc.scalar.copy(payl_t[:, j, c0:c0 + NC], psum[:, :, j])
                nc.scalar.copy(ylb_t[:, c0:c0 + NC], psum[:, :, 5])
                if debug_taps and ci == 0:
                    nc.sync.dma_start(doh_d[0:P, 0:NC * NT],
                                      oh2[:].rearrange("p n t -> p (n t)"))
                    nc.sync.dma_start(
                        dohT_d[0:NT, :],
                        ohTs[0:NT, :, :].rearrange("p n q -> p (n q)"))

                # ---- DVE: focal row sums + label-column select ----
                scb = fspool.tile([P, NC, C], BF16, tag="scb", name="scb")
                nc.scalar.copy(scb[:], sc[:])
                p2 = fspool.tile([P, NC, C], BF16, tag="p2", name="p2")
                nc.vector.tensor_mul(p2[:], pb[:], pb[:])
                nc.vector.tensor_mul(p2[:], p2[:], lnp[:])
                tree_last(p2, p2, C, ALU.add)
                nc.scalar.copy(rs0_t[:, c0:c0 + NC], p2[:, :, 0:1].squeeze(2))
                syp = fspool.tile([P, NC, C], BF16, tag="syp", name="syp")
                yle = ylb_t[:, c0:c0 + NC].unsqueeze(2) \
                    .broadcast_to([P, NC, C])
                nc.vector.tensor_tensor(syp[:], cie, yle, ALU.is_equal)
                nc.vector.tensor_mul(syp[:], syp[:], scb[:])
                tree_last(syp, syp, C, ALU.add)
                nc.scalar.copy(sy_t[:, c0:c0 + NC], syp[:, :, 0:1].squeeze(2))

            # ---------- objectness BCE ----------
            l0_t = ppool.tile([P, NPP], F32, tag="l0", name="l0")
            nc.scalar.activation(l0_t[:], obj_t[:], ACTF.Ln, bias=1.0,
                                 scale=-1.0)
            l1_t = ppool.tile([P, NPP], F32, tag="l1", name="l1")
            nc.scalar.activation(l1_t[:], obj_t[:], ACTF.Ln)
            nc.vector.tensor_single_scalar(l1_t[:], l1_t[:], -100.0, ALU.max)
            nc.vector.tensor_reduce(part_t[:, 1:2], l0_t[:], AX.X, ALU.add)
            nc.vector.tensor_sub(l1_t[:], l1_t[:], l0_t[:])   # logit diff

            # ---------- pos mask + masked sums ----------
            nc.vector.tensor_single_scalar(posf_t[:], mxf_t[:], THIRD,
                                           ALU.is_ge)
            nc.vector.tensor_reduce(part_t[:, 0:1], posf_t[:], AX.X, ALU.add)
            nc.vector.tensor_mul(l1_t[:], l1_t[:], posf_t[:, 0:NPP])
            nc.vector.tensor_reduce(part_t[:, 2:3], l1_t[:], AX.X, ALU.add)

            # ---------- focal correction planes ----------
            py_t = plane("py")
            nc.scalar.activation(py_t[:], sy_t[:], ACTF.Sigmoid)
            lnpy_t = plane("lnpy")
            nc.scalar.activation(lnpy_t[:], py_t[:], ACTF.Ln)      # = -spny
            ly_t = plane("ly")
            nc.scalar.activation(ly_t[:], py_t[:], ACTF.Ln, bias=1.0,
                                 scale=-1.0)                       # ln(1-py)
            qy_t = plane("qy")
            nc.vector.tensor_scalar(qy_t[:], py_t[:], -1.0, 1.0, ALU.mult,
                                    ALU.add)
            nc.vector.tensor_mul(qy_t[:], qy_t[:], qy_t[:])
            g1_t = plane("g1")
            nc.vector.scalar_tensor_tensor(g1_t[:], lnpy_t[:], -0.25, qy_t[:],
                                           ALU.mult, ALU.mult)     # g1y
            py2_t = plane("py2")
            nc.vector.tensor_mul(py2_t[:], py_t[:], py_t[:])
            g0_t = plane("g0")
            nc.vector.scalar_tensor_tensor(g0_t[:], py2_t[:], -0.75, ly_t[:],
                                           ALU.mult, ALU.mult)     # g0y
            nc.vector.tensor_sub(g1_t[:], g1_t[:], g0_t[:])        # corr
            row_t = plane("row")
            nc.vector.scalar_tensor_tensor(row_t[:], rs0_t[:], -0.75, g1_t[:],
                                           ALU.mult, ALU.add)
            nc.vector.tensor_mul(row_t[:], row_t[:], posf_t[:])
            nc.vector.tensor_reduce(part_t[:, 4:5], row_t[:], AX.X, ALU.add)

            # ---------- GIoU planes (f32) ----------
            thxM = payl_t[:, 0, :]
            tlxM = payl_t[:, 1, :]
            thyM = payl_t[:, 2, :]
            tlyM = payl_t[:, 3, :]
            taeM = payl_t[:, 4, :]
            sM_t = plane("sM")
            nc.vector.tensor_tensor(sM_t[:], af_t[:, 4, :], taeM, ALU.add)
            ipM_t = plane("ipM")
            nc.vector.tensor_mul(ipM_t[:], mxf_t[:], sM_t[:])
            un_t = plane("un")
            nc.vector.tensor_sub(un_t[:], sM_t[:], ipM_t[:])   # union + 1e-6
            ru_t = plane("ru")
            nc.vector.reciprocal_approx_fast(ru_t[:], un_t[:])
            iouM_t = plane("iouM")
            nc.vector.tensor_mul(iouM_t[:], ipM_t[:], ru_t[:])
            ex_t = plane("ex")
            sc2_t = plane("sc2")
            nc.vector.tensor_tensor(ex_t[:], af_t[:, 0, :], thxM, ALU.max)
            nc.vector.tensor_tensor(sc2_t[:], af_t[:, 1, :], tlxM, ALU.min)
            nc.vector.tensor_sub(ex_t[:], ex_t[:], sc2_t[:])
            ey_t = plane("ey")
            nc.vector.tensor_tensor(ey_t[:], af_t[:, 2, :], thyM, ALU.max)
            nc.vector.tensor_tensor(sc2_t[:], af_t[:, 3, :], tlyM, ALU.min)
            nc.vector.tensor_sub(ey_t[:], ey_t[:], sc2_t[:])
            nc.vector.tensor_mul(ex_t[:], ex_t[:], ey_t[:])    # enclose
            nc.vector.tensor_scalar_add(ex_t[:], ex_t[:], 1e-6)
            nc.vector.tensor_sub(ey_t[:], ex_t[:], un_t[:])    # encl - union
            nc.vector.reciprocal_approx_fast(ex_t[:], ex_t[:])
            nc.vector.tensor_mul(ey_t[:], ey_t[:], ex_t[:])
            nc.vector.tensor_sub(iouM_t[:], iouM_t[:], ey_t[:])  # giou
            nc.vector.tensor_mul(iouM_t[:], iouM_t[:], posf_t[:])
            nc.vector.tensor_scalar(iouM_t[:], iouM_t[:], -1.0, 0.0,
                                    ALU.mult, ALU.add)
            nc.vector.tensor_reduce(part_t[:, 3:4], iouM_t[:], AX.X, ALU.add)

            # ---------- cross-partition reduce + final scalars ----------
            red_t = ppool.tile([P, 8], F32)
            nc.gpsimd.partition_all_reduce(red_t[:], part_t[:], P,
                                           bass_isa.ReduceOp.add)
            r0 = red_t[0:1, :]
            out_t = ppool.tile([1, 8], F32)
            nc.vector.memset(out_t[:], 0.0)
            s1 = ppool.tile([1, 1], F32, tag="s1", name="s1")
            nc.vector.tensor_add(s1[:], r0[:, 1:2], r0[:, 2:3])
            c96 = ppool.tile([1, 1], F32, tag="c96", name="c96")
            nc.vector.memset(c96[:], float(N) * 0.5)
            s2 = ppool.tile([1, 1], F32, tag="s2", name="s2")
            nc.vector.scalar_tensor_tensor(s2[:], r0[:, 0:1], 0.5, c96[:],
                                           ALU.mult, ALU.add)
            nc.vector.scalar_tensor_tensor(out_t[:, 0:1], s1[:], -1.0, s2[:],
                                           ALU.mult, ALU.mult)
            nc.vector.tensor_add(out_t[:, 1:2], r0[:, 0:1], r0[:, 3:4])
            s3 = ppool.tile([1, 1], F32, tag="s3", name="s3")
            nc.vector.tensor_scalar(s3[:], r0[:, 0:1], float(C), 1.0,
                                    ALU.mult, ALU.max)
            nc.vector.reciprocal(s3[:], s3[:])
            nc.vector.tensor_mul(out_t[:, 2:3], r0[:, 4:5], s3[:])
            nc.vector.tensor_copy(out_t[:, 3:4], r0[:, 0:1])
            if debug_taps:
                dp = cpool.tile([P, 6, NPA], F32, tag="dp", name="dp")
                nc.scalar.copy(dp[:, 0:5, :], payl_t[:])
                nc.scalar.copy(dp[:, 5, :], ylb_t[:])
                nc.sync.dma_start(dpayl_d[:], dp[:])
            nc.sync.dma_start(out_d[:], out_t[:])

    nc.compile()
    return nc


def prep_core_inputs(objectness, boxes, class_scores, target_boxes,
                     target_labels):
    """Split full inputs into 8 per-core input maps (host-side precompute)."""
    import ml_dtypes
    bf16 = ml_dtypes.bfloat16
    objf = np.ascontiguousarray(objectness, dtype=np.float32).reshape(B, N)
    boxf = np.ascontiguousarray(boxes, dtype=np.float32).reshape(B, N, 4)
    clsf = np.ascontiguousarray(class_scores, dtype=np.float32).reshape(B, N, C)
    tbs = np.asarray(target_boxes, dtype=np.float32)
    tls = np.asarray(target_labels)

    rife1 = np.repeat((199.0 - np.arange(NT, dtype=np.float32))[:, None],
                      NC, axis=1).reshape(NT * NC)
    rife = np.broadcast_to(rife1[None, :], (P, NT * NC)).astype(bf16)
    rife21 = np.repeat((199.0 - np.arange(NT, dtype=np.float32))[None, :],
                       NC, axis=0).reshape(NC * NT)
    rife2 = np.broadcast_to(rife21[None, :], (P, NC * NT)).astype(bf16)
    cif = np.broadcast_to(np.arange(C, dtype=np.float32)[None, :],
                          (P, C)).astype(bf16)
    ident = np.eye(P, dtype=np.float32).astype(bf16)

    in_maps = []
    for b in range(B):
        obj = objf[b].reshape(P, NPP)
        bx = boxf[b].reshape(P, NPP, 4)
        cx, cy, w, h = bx[..., 0], bx[..., 1], bx[..., 2], bx[..., 3]
        af = np.zeros((P, 5, NPA), dtype=np.float32)
        af[:, 0, :NPP] = cx + 0.5 * w
        af[:, 1, :NPP] = cx - 0.5 * w
        af[:, 2, :NPP] = cy + 0.5 * h
        af[:, 3, :NPP] = cy - 0.5 * h
        af[:, 4, :NPP] = w * h
        af[:, 0, NPP:] = -20.0   # dummy anchors: zero intersection
        af[:, 1, NPP:] = -19.0
        af[:, 2, NPP:] = -20.0
        af[:, 3, NPP:] = -19.0
        af[:, 4, NPP:] = 1.0
        ab = af.astype(bf16)
        cls = np.zeros((P, NPA, C), dtype=np.float32)
        cls[:, :NPP, :] = clsf[b].reshape(P, NPP, C)
        cls = cls.reshape(P, NPA * C)
        tb = tbs[b]
        pay = np.stack([tb[:, 0] + 0.5 * tb[:, 2],
                        tb[:, 0] - 0.5 * tb[:, 2],
                        tb[:, 1] + 0.5 * tb[:, 3],
                        tb[:, 1] - 0.5 * tb[:, 3],
                        tb[:, 2] * tb[:, 3] + 1e-6,
                        tls[b].astype(np.float32)], axis=1)  # [NT, 6]
        te1 = np.repeat(pay[:, :, None], NC, axis=2).reshape(NT, NPAY * NC)
        teall = np.broadcast_to(te1[None, :, :],
                                (P, NT, NPAY * NC)).astype(bf16)
        tab = np.zeros((P, 8), dtype=np.float32)
        tab[:NT, :NPAY] = pay
        in_maps.append({"obj": obj, "af": af, "ab": np.ascontiguousarray(ab),
                        "cls": cls,
                        "teall": np.ascontiguousarray(teall),
                        "rife": np.ascontiguousarray(rife),
                        "rife2": np.ascontiguousarray(rife2),
                        "tab": tab.astype(bf16),
                        "cif": np.ascontiguousarray(cif),
                        "ident": np.ascontiguousarray(ident)})
    return in_maps


def combine_outputs(outs):
    """outs: list of 8 per-core [1,8] arrays -> scalar loss."""
    o = np.stack([np.asarray(x).reshape(8) for x in outs])  # [8, 8]
    obj_terms, bb_sums, cl_sums, pcs = o[:, 0], o[:, 1], o[:, 2], o[:, 3]
    num_pos = max(float(pcs.sum()), 1.0)
    loss = (np.float32(obj_terms.sum()) / np.float32(B)
            + np.float32(5.0) * np.float32(bb_sums.sum()) / np.float32(num_pos)
            + np.float32(cl_sums.sum()) / np.float32(B))
    return np.float32(loss)


_NC_CACHE = {}


def kernel(objectness, boxes, class_scores, target_boxes, target_labels):
    from concourse.bass_utils import run_bass_kernel_spmd
    if "nc" not in _NC_CACHE:
        _NC_CACHE["nc"] = build_kernel()
    nc = _NC_CACHE["nc"]
    in_maps = prep_core_inputs(objectness, boxes, class_scores,
                               target_boxes, target_labels)
    res = run_bass_kernel_spmd(nc, in_maps, core_ids=list(range(B)))
    outs = [res.results[b]["out"] for b in range(B)]
    return combine_outputs(outs)


# revision 14
# speedup vs baseline: 1.8762x; 1.1612x over previous
"""Trainium2 Bass kernel for nn_DetectionLoss (B=8, A=3, H=W=80, C=80, M=100).

Data-parallel: image b -> core b (8 cores). Each core computes its image's
partial sums [pos_cnt, sum_l0, sum_posf*l1d, -sum_giou*posf, sum_row*posf];
host combines.

v3 design:
- Anchors padded 150->160 per partition (dummy anchors produce ~zero
  intersection -> never positive); chunk width NC=32, 5 chunks.
- IoU pair phase on DVE in [P,NT,NC] bf16. Ranking in ln-domain:
  g' = ln(ip) - ln(S) (monotone in iou = ip/(S-ip)); the two Ln sweeps run
  in-place on the Scalar engine, removing the f32 reciprocal chain from DVE.
  Relu clamps at 1e-18 keep ln(ip) finite.
- 1-chunk software pipeline: DVE does pair(i), then rank(i-1), then focal
  phases, so scalar Ln results and PE select results are consumed one chunk
  later and never stall DVE.
- Matched-payload select on the PE: per chunk, 32 PE transposes turn the
  exclusive one-hot (rife tie-break) into [t, p] layout in PSUM, one scalar
  copy stages it to SBUF, and 32 tiny matmuls against the per-target payload
  table ([thx,tlx,thy,tly,tae,label]) produce matched payloads per anchor.
- Focal loss: sigmoid/ln(1-p) sweeps on Scalar; p^2*ln(1-p) row sums and the
  label-column (s_y) select on DVE in bf16; correction from plane ACT ops.
- Cross-partition reduce via gpsimd.partition_all_reduce.
"""
import numpy as np

import concourse.bass as bass
import concourse.bacc as bacc
import concourse.mybir as mybir
import concourse.tile as tile
from concourse import bass_isa

F32 = mybir.dt.float32
BF16 = mybir.dt.bfloat16
ALU = mybir.AluOpType
ACTF = mybir.ActivationFunctionType
AX = mybir.AxisListType

P = 128          # partitions
NPP = 150        # real anchors per partition
NPA = 160        # padded anchors per partition
N = P * NPP      # 19200 real anchors
NT = 100         # targets
C = 80           # classes
NC = 32          # anchor chunk width
NCH = NPA // NC  # 5 chunks
NPAY = 6         # payload cols: thx, tlx, thy, tly, tae, label
B = 8
LN_THIRD = float(np.log(1.0 / 3.0))


def build_kernel():
    nc = bacc.Bacc(None, target_bir_lowering=False, debug=False)

    obj_d = nc.dram_tensor("obj", [P, NPP], F32, kind="ExternalInput")
    af_d = nc.dram_tensor("af", [P, 5, NPA], F32, kind="ExternalInput")
    ab_d = nc.dram_tensor("ab", [P, 5, NPA], BF16, kind="ExternalInput")
    cls_d = nc.dram_tensor("cls", [P, NPA * C], F32, kind="ExternalInput")
    teall_d = nc.dram_tensor("teall", [P, 5, NT * NC], BF16,
                             kind="ExternalInput")
    rife_d = nc.dram_tensor("rife", [P, NT * NC], BF16, kind="ExternalInput")
    rife2_d = nc.dram_tensor("rife2", [P, NC * NT], BF16, kind="ExternalInput")
    tab_d = nc.dram_tensor("tab", [P, 8], BF16, kind="ExternalInput")
    cif_d = nc.dram_tensor("cif", [P, C], BF16, kind="ExternalInput")
    ident_d = nc.dram_tensor("ident", [P, P], BF16, kind="ExternalInput")
    out_d = nc.dram_tensor("out", [1, 8], F32, kind="ExternalOutput")

    with nc.allow_low_precision("bf16 iou/focal phases are tolerance-analyzed"), \
         tile.TileContext(nc) as tc:
        with tc.tile_pool(name="const", bufs=1) as cpool, \
             tc.tile_pool(name="planes", bufs=1) as ppool, \
             tc.tile_pool(name="iou", bufs=1) as ipool, \
             tc.tile_pool(name="cross", bufs=2) as xpool, \
             tc.tile_pool(name="oh", bufs=1) as opool, \
             tc.tile_pool(name="foc", bufs=2) as fpool, \
             tc.tile_pool(name="focs", bufs=1) as fspool, \
             tc.tile_pool(name="psum", bufs=2, space="PSUM") as qpool:

            def plane(tag, dt=F32):
                return ppool.tile([P, NPA], dt, tag=tag, name=tag)

            # ---------- resident loads (teall slot 0 first: unblocks DVE) --
            teall_t = cpool.tile([P, 5, NT, NC], BF16)
            for j in range(5):
                nc.sync.dma_start(
                    teall_t[:, j, :, :].rearrange("p t n -> p (t n)"),
                    teall_d[:, j, :])
            ab_t = cpool.tile([P, 5, NPA], BF16)
            nc.sync.dma_start(ab_t[:], ab_d[:])
            rife_t = cpool.tile([P, NT, NC], BF16)
            nc.sync.dma_start(
                rife_t[:].rearrange("p t n -> p (t n)"), rife_d[:])
            rife2_t = cpool.tile([P, NC, NT], BF16)
            nc.sync.dma_start(
                rife2_t[:].rearrange("p n t -> p (n t)"), rife2_d[:])
            tab_t = cpool.tile([P, 8], BF16)
            nc.sync.dma_start(tab_t[:], tab_d[:])
            cif_t = cpool.tile([P, C], BF16)
            nc.sync.dma_start(cif_t[:], cif_d[:])
            ident_t = cpool.tile([P, P], BF16)
            nc.sync.dma_start(ident_t[:], ident_d[:])
            af_t = cpool.tile([P, 5, NPA], F32)
            nc.sync.dma_start(af_t[:], af_d[:])
            obj_t = cpool.tile([P, NPP], F32)
            nc.sync.dma_start(obj_t[:], obj_d[:])

            part_t = ppool.tile([P, 8], F32)
            nc.vector.memset(part_t[:, 5:8], 0.0)

            mxf_t = plane("mxf")                    # max ln(g) per anchor
            payl_t = cpool.tile([P, 5, NPA], F32)   # matched payload planes
            ylb_t = plane("ylb", BF16)              # matched label (bf16)
            rs0_t = plane("rs0")                    # sum_c p^2 ln(1-p)
            sy_t = plane("sy")                      # logit at label column
            posf_t = plane("posf")

            cls3 = cls_d[:].rearrange("p (n c) -> p n c", c=C)

            def tree1(scratch, src, w, op):
                first = True
                while w > 1:
                    h = w // 2
                    s = src if first else scratch
                    nc.vector.tensor_tensor(scratch[:, 0:h], s[:, 0:h],
                                            s[:, h:2 * h], op)
                    if w % 2:
                        nc.vector.tensor_tensor(scratch[:, 0:1],
                                                scratch[:, 0:1],
                                                s[:, w - 1:w], op)
                    first = False
                    w = h
                return scratch

            def tree_last(scratch, src, w, op):
                first = True
                while w > 1:
                    h = w // 2
                    s = src if first else scratch
                    nc.vector.tensor_tensor(scratch[:, :, 0:h], s[:, :, 0:h],
                                            s[:, :, h:2 * h], op)
                    if w % 2:
                        nc.vector.tensor_tensor(scratch[:, :, 0:1],
                                                scratch[:, :, 0:1],
                                                s[:, :, w - 1:w], op)
                    first = False
                    w = h
                return scratch

            def aexp(j, c0):
                return ab_t[:, j, c0:c0 + NC].unsqueeze(1) \
                    .broadcast_to([P, NT, NC])

            cie = cif_t[:].unsqueeze(1).broadcast_to([P, NC, C])

            # pair scratch (reused by rank: DVE is serial)
            ta = ipool.tile([P, NT, NC], BF16, tag="ta", name="ta")
            tb = ipool.tile([P, NT, NC], BF16, tag="tb", name="tb")
            rc1 = ipool.tile([P, NT, NC], BF16, tag="rc1", name="rc1")

            ipb = [None] * NCH   # ln(ip) tiles (cross-stage, bufs=2)
            sbb = [None] * NCH   # ln(S) tiles
            scb = [None] * NCH   # bf16 logits
            pbt = [None] * NCH
            lnpt = [None] * NCH

            def pair(i):
                c0 = i * NC
                sc = fpool.tile([P, NC, C], F32, tag="sc", name="sc", bufs=2)
                nc.sync.dma_start(sc[:], cls3[:, c0:c0 + NC, :])
                pb = fpool.tile([P, NC, C], BF16, tag="pb", name="pb")
                nc.scalar.activation(pb[:], sc[:], ACTF.Sigmoid)
                lnp = fpool.tile([P, NC, C], BF16, tag="lnp", name="lnp")
                nc.scalar.activation(lnp[:], pb[:], ACTF.Ln, bias=1.0,
                                     scale=-1.0)
                pbt[i], lnpt[i] = pb, lnp
                scbt = fspool.tile([P, NC, C], BF16, tag="scb", name="scb",
                                   bufs=2)
                nc.scalar.copy(scbt[:], sc[:])
                scb[i] = scbt

                ip = xpool.tile([P, NT, NC], BF16, tag="ipb", name="ipb")
                sb = xpool.tile([P, NT, NC], BF16, tag="sbb", name="sbb")
                nc.vector.tensor_tensor(ta[:], aexp(0, c0),
                                        teall_t[:, 0, :, :], ALU.min)  # hx
                nc.vector.tensor_tensor(tb[:], aexp(1, c0),
                                        teall_t[:, 1, :, :], ALU.max)  # lx
                nc.vector.tensor_sub(ta[:], ta[:], tb[:])              # wx
                nc.vector.tensor_single_scalar(ip[:], ta[:], 1e-18, ALU.max)
                nc.vector.tensor_tensor(ta[:], aexp(2, c0),
                                        teall_t[:, 2, :, :], ALU.min)  # hy
                nc.vector.tensor_tensor(tb[:], aexp(3, c0),
                                        teall_t[:, 3, :, :], ALU.max)  # ly
                nc.vector.tensor_sub(ta[:], ta[:], tb[:])              # wy
                nc.vector.tensor_single_scalar(tb[:], ta[:], 1e-18, ALU.max)
                nc.vector.tensor_mul(ip[:], ip[:], tb[:])              # ip>0
                nc.vector.tensor_tensor(sb[:], teall_t[:, 4, :, :],
                                        aexp(4, c0), ALU.add)          # S
                # ln sweeps in place on scalar (consumed by rank next chunk)
                nc.scalar.activation(ip[:], ip[:], ACTF.Ln)
                nc.scalar.activation(sb[:], sb[:], ACTF.Ln)
                ipb[i], sbb[i] = ip, sb

                # focal row sums (pb/lnp ready; independent of ranking)
                p2 = fspool.tile([P, NC, C], BF16, tag="p2", name="p2")
                nc.vector.tensor_mul(p2[:], pb[:], pb[:])
                nc.vector.tensor_mul(p2[:], p2[:], lnp[:])
                tree_last(p2, p2, C, ALU.add)
                nc.scalar.copy(rs0_t[:, c0:c0 + NC], p2[:, :, 0:1].squeeze(2))

            def rank(j):
                c0 = j * NC
                nc.vector.tensor_sub(rc1[:], ipb[j][:], sbb[j][:])     # ln g
                mx = tree1(ta, rc1, NT, ALU.max)
                mxe = mx[:, 0:1, :].broadcast_to([P, NT, NC])
                nc.vector.tensor_tensor(tb[:], rc1[:], mxe, ALU.is_equal)
                nc.vector.tensor_mul(tb[:], tb[:], rife_t[:])          # rsel
                rmx = tree1(rc1, tb, NT, ALU.max)
                nc.scalar.copy(mxf_t[:, c0:c0 + NC], mx[:, 0:1, :].squeeze(1))
                oh2 = opool.tile([P, NC, NT], BF16, tag="oh2", name="oh2")
                rme2 = rmx[:, 0, :].unsqueeze(2).broadcast_to([P, NC, NT])
                nc.vector.tensor_tensor(oh2[:], rife2_t[:], rme2,
                                        ALU.is_equal)                  # 1-hot
                # PE: transpose one-hot, stage to SBUF, select payloads
                psT = qpool.tile([P, NC, P], BF16, tag="psT", name="psT",
                                 bufs=1)
                for n in range(NC):
                    nc.tensor.transpose(psT[0:NT, n, :], oh2[:, n, :],
                                        ident_t[:])
                ohTs = opool.tile([P, NC, P], BF16, tag="ohTs",
                                  name="ohTs", bufs=1)
                nc.scalar.copy(ohTs[0:NT, :, :], psT[0:NT, :, :])
                psum = qpool.tile([P, NC, NPAY], F32, tag="ps", name="ps")
                for n in range(NC):
                    nc.tensor.matmul(psum[:, n, :], ohTs[0:NT, n, :],
                                     tab_t[0:NT, 0:NPAY])
                for k in range(5):
                    nc.scalar.copy(payl_t[:, k, c0:c0 + NC], psum[:, :, k])
                nc.scalar.copy(ylb_t[:, c0:c0 + NC], psum[:, :, 5])

            def phaseB(j):
                c0 = j * NC
                yle = ylb_t[:, c0:c0 + NC].unsqueeze(2) \
                    .broadcast_to([P, NC, C])
                syp = fspool.tile([P, NC, C], BF16, tag="syp", name="syp")
                nc.vector.tensor_tensor(syp[:], cie, yle, ALU.is_equal)
                nc.vector.tensor_mul(syp[:], syp[:], scb[j][:])
                tree_last(syp, syp, C, ALU.add)
                nc.scalar.copy(sy_t[:, c0:c0 + NC], syp[:, :, 0:1].squeeze(2))

            # ---------- pipelined main loop ----------
            for i in range(NCH):
                pair(i)
                if i >= 1:
                    rank(i - 1)
                if i >= 2:
                    phaseB(i - 2)
            rank(NCH - 1)
            phaseB(NCH - 2)
            phaseB(NCH - 1)

            # ---------- objectness BCE (Ln table still loaded) -------------
            l0_t = ppool.tile([P, NPP], F32, tag="l0", name="l0")
            nc.scalar.activation(l0_t[:], obj_t[:], ACTF.Ln, bias=1.0,
                                 scale=-1.0)
            l1_t = ppool.tile([P, NPP], F32, tag="l1", name="l1")
            nc.scalar.activation(l1_t[:], obj_t[:], ACTF.Ln)
            nc.vector.tensor_single_scalar(l1_t[:], l1_t[:], -100.0, ALU.max)
            nc.vector.tensor_reduce(part_t[:, 1:2], l0_t[:], AX.X, ALU.add)
            nc.vector.tensor_sub(l1_t[:], l1_t[:], l0_t[:])   # logit diff

            # ---------- pos mask + masked sums ----------
            nc.vector.tensor_single_scalar(posf_t[:], mxf_t[:], LN_THIRD,
                                           ALU.is_ge)
            nc.vector.tensor_reduce(part_t[:, 0:1], posf_t[:], AX.X, ALU.add)
            nc.vector.tensor_mul(l1_t[:], l1_t[:], posf_t[:, 0:NPP])
            nc.vector.tensor_reduce(part_t[:, 2:3], l1_t[:], AX.X, ALU.add)

            # ---------- focal correction planes ----------
            py_t = plane("py")
            nc.scalar.activation(py_t[:], sy_t[:], ACTF.Sigmoid)
            lnpy_t = plane("lnpy")
            nc.scalar.activation(lnpy_t[:], py_t[:], ACTF.Ln)      # = -spny
            ly_t = plane("ly")
            nc.scalar.activation(ly_t[:], py_t[:], ACTF.Ln, bias=1.0,
                                 scale=-1.0)                       # ln(1-py)
            qy_t = plane("qy")
            nc.vector.tensor_scalar(qy_t[:], py_t[:], -1.0, 1.0, ALU.mult,
                                    ALU.add)
            nc.vector.tensor_mul(qy_t[:], qy_t[:], qy_t[:])
            g1_t = plane("g1")
            nc.vector.scalar_tensor_tensor(g1_t[:], lnpy_t[:], -0.25, qy_t[:],
                                           ALU.mult, ALU.mult)     # g1y
            py2_t = plane("py2")
            nc.vector.tensor_mul(py2_t[:], py_t[:], py_t[:])
            g0_t = plane("g0")
            nc.vector.scalar_tensor_tensor(g0_t[:], py2_t[:], -0.75, ly_t[:],
                                           ALU.mult, ALU.mult)     # g0y
            nc.vector.tensor_sub(g1_t[:], g1_t[:], g0_t[:])        # corr
            row_t = plane("row")
            nc.vector.scalar_tensor_tensor(row_t[:], rs0_t[:], -0.75, g1_t[:],
                                           ALU.mult, ALU.add)
            nc.vector.tensor_mul(row_t[:], row_t[:], posf_t[:])
            nc.vector.tensor_reduce(part_t[:, 4:5], row_t[:], AX.X, ALU.add)

            # ---------- GIoU planes (f32) ----------
            gme_t = plane("gme")
            nc.scalar.activation(gme_t[:], mxf_t[:], ACTF.Exp)     # g_m
            thxM = payl_t[:, 0, :]
            tlxM = payl_t[:, 1, :]
            thyM = payl_t[:, 2, :]
            tlyM = payl_t[:, 3, :]
            taeM = payl_t[:, 4, :]
            sM_t = plane("sM")
            nc.vector.tensor_tensor(sM_t[:], af_t[:, 4, :], taeM, ALU.add)
            ipM_t = plane("ipM")
            nc.vector.tensor_mul(ipM_t[:], gme_t[:], sM_t[:])
            un_t = plane("un")
            nc.vector.tensor_sub(un_t[:], sM_t[:], ipM_t[:])   # union + 1e-6
            ru_t = plane("ru")
            nc.vector.reciprocal_approx_fast(ru_t[:], un_t[:])
            iouM_t = plane("iouM")
            nc.vector.tensor_mul(iouM_t[:], ipM_t[:], ru_t[:])
            ex_t = plane("ex")
            sc2_t = plane("sc2")
            nc.vector.tensor_tensor(ex_t[:], af_t[:, 0, :], thxM, ALU.max)
            nc.vector.tensor_tensor(sc2_t[:], af_t[:, 1, :], tlxM, ALU.min)
            nc.vector.tensor_sub(ex_t[:], ex_t[:], sc2_t[:])
            ey_t = plane("ey")
            nc.vector.tensor_tensor(ey_t[:], af_t[:, 2, :], thyM, ALU.max)
            nc.vector.tensor_tensor(sc2_t[:], af_t[:, 3, :], tlyM, ALU.min)
            nc.vector.tensor_sub(ey_t[:], ey_t[:], sc2_t[:])
            nc.vector.tensor_mul(ex_t[:], ex_t[:], ey_t[:])    # enclose
            nc.vector.tensor_scalar_add(ex_t[:], ex_t[:], 1e-6)
            nc.vector.tensor_sub(ey_t[:], ex_t[:], un_t[:])    # encl - union
            nc.vector.reciprocal_approx_fast(ex_t[:], ex_t[:])
            nc.vector.tensor_mul(ey_t[:], ey_t[:], ex_t[:])
            nc.vector.tensor_sub(iouM_t[:], iouM_t[:], ey_t[:])  # giou
            nc.vector.tensor_mul(iouM_t[:], iouM_t[:], posf_t[:])
            nc.vector.tensor_scalar(iouM_t[:], iouM_t[:], -1.0, 0.0,
                                    ALU.mult, ALU.add)
            nc.vector.tensor_reduce(part_t[:, 3:4], iouM_t[:], AX.X, ALU.add)

            # ---------- cross-partition reduce + final scalars ----------
            red_t = ppool.tile([P, 8], F32)
            nc.gpsimd.partition_all_reduce(red_t[:], part_t[:], P,
                                           bass_isa.ReduceOp.add)
            r0 = red_t[0:1, :]
            out_t = ppool.tile([1, 8], F32)
            nc.vector.memset(out_t[:], 0.0)
            s1 = ppool.tile([1, 1], F32, tag="s1", name="s1")
            nc.vector.tensor_add(s1[:], r0[:, 1:2], r0[:, 2:3])
            c96 = ppool.tile([1, 1], F32, tag="c96", name="c96")
            nc.vector.memset(c96[:], float(N) * 0.5)
            s2 = ppool.tile([1, 1], F32, tag="s2", name="s2")
            nc.vector.scalar_tensor_tensor(s2[:], r0[:, 0:1], 0.5, c96[:],
                                           ALU.mult, ALU.add)
            nc.vector.scalar_tensor_tensor(out_t[:, 0:1], s1[:], -1.0, s2[:],
                                           ALU.mult, ALU.mult)
            nc.vector.tensor_add(out_t[:, 1:2], r0[:, 0:1], r0[:, 3:4])
            s3 = ppool.tile([1, 1], F32, tag="s3", name="s3")
            nc.vector.tensor_scalar(s3[:], r0[:, 0:1], float(C), 1.0,
                                    ALU.mult, ALU.max)
            nc.vector.reciprocal(s3[:], s3[:])
            nc.vector.tensor_mul(out_t[:, 2:3], r0[:, 4:5], s3[:])
            nc.vector.tensor_copy(out_t[:, 3:4], r0[:, 0:1])
            nc.sync.dma_start(out_d[:], out_t[:])

    nc.compile()
    return nc


def prep_core_inputs(objectness, boxes, class_scores, target_boxes,
                     target_labels):
    """Split full inputs into 8 per-core input maps (host-side precompute)."""
    import ml_dtypes
    bf16 = ml_dtypes.bfloat16
    objf = np.ascontiguousarray(objectness, dtype=np.float32).reshape(B, N)
    boxf = np.ascontiguousarray(boxes, dtype=np.float32).reshape(B, N, 4)
    clsf = np.ascontiguousarray(class_scores, dtype=np.float32).reshape(B, N, C)
    tbs = np.asarray(target_boxes, dtype=np.float32)
    tls = np.asarray(target_labels)

    rife1 = np.repeat((199.0 - np.arange(NT, dtype=np.float32))[:, None],
                      NC, axis=1).reshape(NT * NC)
    rife = np.broadcast_to(rife1[None, :], (P, NT * NC)).astype(bf16)
    rife21 = np.repeat((199.0 - np.arange(NT, dtype=np.float32))[None, :],
                       NC, axis=0).reshape(NC * NT)
    rife2 = np.broadcast_to(rife21[None, :], (P, NC * NT)).astype(bf16)
    cif = np.broadcast_to(np.arange(C, dtype=np.float32)[None, :],
                          (P, C)).astype(bf16)
    ident = np.eye(P, dtype=np.float32).astype(bf16)

    in_maps = []
    for b in range(B):
        obj = objf[b].reshape(P, NPP)
        bx = boxf[b].reshape(P, NPP, 4)
        cx, cy, w, h = bx[..., 0], bx[..., 1], bx[..., 2], bx[..., 3]
        af = np.zeros((P, 5, NPA), dtype=np.float32)
        af[:, 0, :NPP] = cx + 0.5 * w
        af[:, 1, :NPP] = cx - 0.5 * w
        af[:, 2, :NPP] = cy + 0.5 * h
        af[:, 3, :NPP] = cy - 0.5 * h
        af[:, 4, :NPP] = w * h
        af[:, 0, NPP:] = -20.0   # dummy anchors: zero intersection
        af[:, 1, NPP:] = -19.0
        af[:, 2, NPP:] = -20.0
        af[:, 3, NPP:] = -19.0
        af[:, 4, NPP:] = 1.0
        ab = af.astype(bf16)
        cls = np.zeros((P, NPA, C), dtype=np.float32)
        cls[:, :NPP, :] = clsf[b].reshape(P, NPP, C)
        cls = cls.reshape(P, NPA * C)
        tb = tbs[b]
        pay = np.stack([tb[:, 0] + 0.5 * tb[:, 2],
                        tb[:, 0] - 0.5 * tb[:, 2],
                        tb[:, 1] + 0.5 * tb[:, 3],
                        tb[:, 1] - 0.5 * tb[:, 3],
                        tb[:, 2] * tb[:, 3] + 1e-6,
                        tls[b].astype(np.float32)], axis=1)  # [NT, 6]
        # teall: slot-major [5, NT, NC] (payload value broadcast along NC)
        te1 = np.repeat(pay[:, 0:5].T[:, :, None], NC, axis=2)  # [5, NT, NC]
        teall = np.broadcast_to(te1.reshape(1, 5, NT * NC),
                                (P, 5, NT * NC)).astype(bf16)
        tab = np.zeros((P, 8), dtype=np.float32)
        tab[:NT, :NPAY] = pay
        in_maps.append({"obj": obj, "af": af, "ab": np.ascontiguousarray(ab),
                        "cls": cls,
                        "teall": np.ascontiguousarray(teall),
                        "rife": np.ascontiguousarray(rife),
                        "rife2": np.ascontiguousarray(rife2),
                        "tab": tab.astype(bf16),
                        "cif": np.ascontiguousarray(cif),
                        "ident": np.ascontiguousarray(ident)})
    return in_maps


def combine_outputs(outs):
    """outs: list of 8 per-core [1,8] arrays -> scalar loss."""
    o = np.stack([np.asarray(x).reshape(8) for x in outs])  # [8, 8]
    obj_terms, bb_sums, cl_sums, pcs = o[:, 0], o[:, 1], o[:, 2], o[:, 3]
    num_pos = max(float(pcs.sum()), 1.0)
    loss = (np.float32(obj_terms.sum()) / np.float32(B)
            + np.float32(5.0) * np.float32(bb_sums.sum()) / np.float32(num_pos)
            + np.float32(cl_sums.sum()) / np.float32(B))
    return np.float32(loss)


_NC_CACHE = {}


def kernel(objectness, boxes, class_scores, target_boxes, target_labels):
    from concourse.bass_utils import run_bass_kernel_spmd
    if "nc" not in _NC_CACHE:
        _NC_CACHE["nc"] = build_kernel()
    nc = _NC_CACHE["nc"]
    in_maps = prep_core_inputs(objectness, boxes, class_scores,
                               target_boxes, target_labels)
    res = run_bass_kernel_spmd(nc, in_maps, core_ids=list(range(B)))
    outs = [res.results[b]["out"] for b in range(B)]
    return combine_outputs(outs)


# revision 15
# speedup vs baseline: 2.0201x; 1.0767x over previous
"""Trainium2 Bass kernel for nn_DetectionLoss (B=8, A=3, H=W=80, C=80, M=100).

Data-parallel: image b -> core b (8 cores). Each core computes its image's
partial sums [pos_cnt, sum_l0, sum_posf*l1d, -sum_giou*posf, sum_row*posf];
host combines.

v3 design:
- Anchors padded 150->160 per partition (dummy anchors produce ~zero
  intersection -> never positive); chunk width NC=32, 5 chunks.
- IoU pair phase on DVE in [P,NT,NC] bf16. Ranking in ln-domain:
  g' = ln(ip) - ln(S) (monotone in iou = ip/(S-ip)); the two Ln sweeps run
  in-place on the Scalar engine, removing the f32 reciprocal chain from DVE.
  Relu clamps at 1e-18 keep ln(ip) finite.
- 1-chunk software pipeline: DVE does pair(i), then rank(i-1), then focal
  phases, so scalar Ln results and PE select results are consumed one chunk
  later and never stall DVE.
- Matched-payload select on the PE: per chunk, 32 PE transposes turn the
  exclusive one-hot (rife tie-break) into [t, p] layout in PSUM, one scalar
  copy stages it to SBUF, and 32 tiny matmuls against the per-target payload
  table ([thx,tlx,thy,tly,tae,label]) produce matched payloads per anchor.
- Focal loss: sigmoid/ln(1-p) sweeps on Scalar; p^2*ln(1-p) row sums and the
  label-column (s_y) select on DVE in bf16; correction from plane ACT ops.
- Cross-partition reduce via gpsimd.partition_all_reduce.
"""
import numpy as np

import concourse.bass as bass
import concourse.bacc as bacc
import concourse.mybir as mybir
import concourse.tile as tile
from concourse import bass_isa

F32 = mybir.dt.float32
BF16 = mybir.dt.bfloat16
ALU = mybir.AluOpType
ACTF = mybir.ActivationFunctionType
AX = mybir.AxisListType

P = 128          # partitions
NPP = 150        # real anchors per partition
NPA = 160        # padded anchors per partition
N = P * NPP      # 19200 real anchors
NT = 100         # targets
C = 80           # classes
NC = 32          # anchor chunk width
NCH = NPA // NC  # 5 chunks
NPAY = 6         # payload cols: thx, tlx, thy, tly, tae, label
B = 8
LN_THIRD = float(np.log(1.0 / 3.0))


def build_kernel():
    nc = bacc.Bacc(None, target_bir_lowering=False, debug=False)

    obj_d = nc.dram_tensor("obj", [P, NPP], F32, kind="ExternalInput")
    af_d = nc.dram_tensor("af", [P, 5, NPA], F32, kind="ExternalInput")
    ab_d = nc.dram_tensor("ab", [P, 5, NPA], BF16, kind="ExternalInput")
    cls_d = nc.dram_tensor("cls", [P, NPA * C], F32, kind="ExternalInput")
    teall_d = nc.dram_tensor("teall", [P, 5, NT * NC], BF16,
                             kind="ExternalInput")
    rife_d = nc.dram_tensor("rife", [P, NT * NC], BF16, kind="ExternalInput")
    rife2_d = nc.dram_tensor("rife2", [P, NC * NT], BF16, kind="ExternalInput")
    tab_d = nc.dram_tensor("tab", [P, 8], BF16, kind="ExternalInput")
    cif_d = nc.dram_tensor("cif", [P, C], BF16, kind="ExternalInput")
    ident_d = nc.dram_tensor("ident", [P, P], BF16, kind="ExternalInput")
    out_d = nc.dram_tensor("out", [1, 8], F32, kind="ExternalOutput")

    with nc.allow_low_precision("bf16 iou/focal phases are tolerance-analyzed"), \
         tile.TileContext(nc) as tc:
        with tc.tile_pool(name="const", bufs=1) as cpool, \
             tc.tile_pool(name="planes", bufs=1) as ppool, \
             tc.tile_pool(name="iou", bufs=1) as ipool, \
             tc.tile_pool(name="cross", bufs=2) as xpool, \
             tc.tile_pool(name="oh", bufs=1) as opool, \
             tc.tile_pool(name="foc", bufs=2) as fpool, \
             tc.tile_pool(name="focs", bufs=1) as fspool, \
             tc.tile_pool(name="psum", bufs=2, space="PSUM") as qpool:

            def plane(tag, dt=F32):
                return ppool.tile([P, NPA], dt, tag=tag, name=tag)

            # ---------- resident loads (ab + teall slots first) ----------
            ab_t = cpool.tile([P, 5, NPA], BF16)
            nc.sync.dma_start(ab_t[:], ab_d[:])
            teall_t = cpool.tile([P, 5, NT, NC], BF16)
            for j in range(5):
                nc.sync.dma_start(
                    teall_t[:, j, :, :].rearrange("p t n -> p (t n)"),
                    teall_d[:, j, :])
            rife_t = cpool.tile([P, NT, NC], BF16)
            nc.sync.dma_start(
                rife_t[:].rearrange("p t n -> p (t n)"), rife_d[:])
            rife2_t = cpool.tile([P, NC, NT], BF16)
            nc.sync.dma_start(
                rife2_t[:].rearrange("p n t -> p (n t)"), rife2_d[:])
            tab_t = cpool.tile([P, 8], BF16)
            nc.sync.dma_start(tab_t[:], tab_d[:])
            cif_t = cpool.tile([P, C], BF16)
            nc.sync.dma_start(cif_t[:], cif_d[:])
            ident_t = cpool.tile([P, P], BF16)
            nc.sync.dma_start(ident_t[:], ident_d[:])
            af_t = cpool.tile([P, 5, NPA], F32)
            nc.sync.dma_start(af_t[:], af_d[:])
            obj_t = cpool.tile([P, NPP], F32)
            nc.sync.dma_start(obj_t[:], obj_d[:])

            part_t = ppool.tile([P, 8], F32)
            nc.vector.memset(part_t[:, 5:8], 0.0)

            mxf_t = plane("mxf")                    # max ln(g) per anchor
            payl_t = cpool.tile([P, 5, NPA], F32)   # matched payload planes
            ylb_t = plane("ylb", BF16)              # matched label (bf16)
            rs0_t = plane("rs0")                    # sum_c p^2 ln(1-p)
            sy_t = plane("sy")                      # logit at label column
            posf_t = plane("posf")

            cls3 = cls_d[:].rearrange("p (n c) -> p n c", c=C)

            def tree1(scratch, src, w, op):
                first = True
                while w > 1:
                    h = w // 2
                    s = src if first else scratch
                    nc.vector.tensor_tensor(scratch[:, 0:h], s[:, 0:h],
                                            s[:, h:2 * h], op)
                    if w % 2:
                        nc.vector.tensor_tensor(scratch[:, 0:1],
                                                scratch[:, 0:1],
                                                s[:, w - 1:w], op)
                    first = False
                    w = h
                return scratch

            def tree_last(scratch, src, w, op):
                first = True
                while w > 1:
                    h = w // 2
                    s = src if first else scratch
                    nc.vector.tensor_tensor(scratch[:, :, 0:h], s[:, :, 0:h],
                                            s[:, :, h:2 * h], op)
                    if w % 2:
                        nc.vector.tensor_tensor(scratch[:, :, 0:1],
                                                scratch[:, :, 0:1],
                                                s[:, :, w - 1:w], op)
                    first = False
                    w = h
                return scratch

            def aexp(j, c0):
                return ab_t[:, j, c0:c0 + NC].unsqueeze(1) \
                    .broadcast_to([P, NT, NC])

            cie = cif_t[:].unsqueeze(1).broadcast_to([P, NC, C])

            # pair scratch (reused by rank: DVE is serial)
            ta = ipool.tile([P, NT, NC], BF16, tag="ta", name="ta")
            tb = ipool.tile([P, NT, NC], BF16, tag="tb", name="tb")
            rc1 = ipool.tile([P, NT, NC], BF16, tag="rc1", name="rc1")

            ipb = [None] * NCH   # ln(ip) tiles (cross-stage, bufs=2)
            sbb = [None] * NCH   # ln(S) tiles
            scb = [None] * NCH   # bf16 logits
            pbt = [None] * NCH
            lnpt = [None] * NCH

            def pair(i):
                c0 = i * NC
                sc = fpool.tile([P, NC, C], F32, tag="sc", name="sc", bufs=2)
                nc.sync.dma_start(sc[:], cls3[:, c0:c0 + NC, :])
                pb = fpool.tile([P, NC, C], BF16, tag="pb", name="pb")
                nc.scalar.activation(pb[:], sc[:], ACTF.Sigmoid)
                lnp = fpool.tile([P, NC, C], BF16, tag="lnp", name="lnp")
                nc.scalar.activation(lnp[:], pb[:], ACTF.Ln, bias=1.0,
                                     scale=-1.0)
                pbt[i], lnpt[i] = pb, lnp
                scbt = fspool.tile([P, NC, C], BF16, tag="scb", name="scb",
                                   bufs=2)
                nc.scalar.copy(scbt[:], sc[:])
                scb[i] = scbt

                ip = xpool.tile([P, NT, NC], BF16, tag="ipb", name="ipb")
                sb = xpool.tile([P, NT, NC], BF16, tag="sbb", name="sbb")
                nc.vector.tensor_tensor(ta[:], aexp(0, c0),
                                        teall_t[:, 0, :, :], ALU.min)  # hx
                nc.vector.tensor_tensor(tb[:], aexp(1, c0),
                                        teall_t[:, 1, :, :], ALU.max)  # lx
                nc.vector.tensor_sub(ta[:], ta[:], tb[:])              # wx
                nc.vector.tensor_single_scalar(ip[:], ta[:], 1e-18, ALU.max)
                nc.vector.tensor_tensor(ta[:], aexp(2, c0),
                                        teall_t[:, 2, :, :], ALU.min)  # hy
                nc.vector.tensor_tensor(tb[:], aexp(3, c0),
                                        teall_t[:, 3, :, :], ALU.max)  # ly
                nc.vector.tensor_sub(ta[:], ta[:], tb[:])              # wy
                nc.vector.tensor_single_scalar(tb[:], ta[:], 1e-18, ALU.max)
                nc.vector.tensor_mul(ip[:], ip[:], tb[:])              # ip>0
                nc.vector.tensor_tensor(sb[:], teall_t[:, 4, :, :],
                                        aexp(4, c0), ALU.add)          # S
                # ln sweeps in place on scalar (consumed by rank next chunk)
                nc.scalar.activation(ip[:], ip[:], ACTF.Ln)
                nc.scalar.activation(sb[:], sb[:], ACTF.Ln)
                ipb[i], sbb[i] = ip, sb

                # focal row sums (pb/lnp ready; independent of ranking)
                p2 = fspool.tile([P, NC, C], BF16, tag="p2", name="p2")
                nc.vector.tensor_mul(p2[:], pb[:], pb[:])
                nc.vector.tensor_mul(p2[:], p2[:], lnp[:])
                tree_last(p2, p2, C, ALU.add)
                nc.scalar.copy(rs0_t[:, c0:c0 + NC], p2[:, :, 0:1].squeeze(2))

            def rank(j):
                c0 = j * NC
                nc.vector.tensor_sub(rc1[:], ipb[j][:], sbb[j][:])     # ln g
                mx = tree1(ta, rc1, NT, ALU.max)
                mxe = mx[:, 0:1, :].broadcast_to([P, NT, NC])
                nc.vector.tensor_tensor(tb[:], rc1[:], mxe, ALU.is_equal)
                nc.vector.tensor_mul(tb[:], tb[:], rife_t[:])          # rsel
                rmx = tree1(rc1, tb, NT, ALU.max)
                nc.scalar.copy(mxf_t[:, c0:c0 + NC], mx[:, 0:1, :].squeeze(1))
                rme2m = fspool.tile([P, NC, NT], BF16, tag="rme2m",
                                    name="rme2m")
                nc.scalar.copy(rme2m[:], rmx[:, 0, :].unsqueeze(2)
                               .broadcast_to([P, NC, NT]))
                oh2 = opool.tile([P, NC, NT], BF16, tag="oh2", name="oh2")
                nc.vector.tensor_tensor(oh2[:], rife2_t[:], rme2m[:],
                                        ALU.is_equal)                  # 1-hot
                # PE: transpose one-hot, stage to SBUF, select payloads
                psT = qpool.tile([P, NC, P], BF16, tag="psT", name="psT",
                                 bufs=1)
                for n in range(NC):
                    nc.tensor.transpose(psT[0:NT, n, :], oh2[:, n, :],
                                        ident_t[:])
                ohTs = opool.tile([P, NC, P], BF16, tag="ohTs",
                                  name="ohTs", bufs=1)
                nc.scalar.copy(ohTs[0:NT, :, :], psT[0:NT, :, :])
                psum = qpool.tile([P, NC, NPAY], F32, tag="ps", name="ps")
                for n in range(NC):
                    nc.tensor.matmul(psum[:, n, :], ohTs[0:NT, n, :],
                                     tab_t[0:NT, 0:NPAY])
                for k in range(5):
                    nc.scalar.copy(payl_t[:, k, c0:c0 + NC], psum[:, :, k])
                nc.scalar.copy(ylb_t[:, c0:c0 + NC], psum[:, :, 5])

            def phaseB(j):
                c0 = j * NC
                ylem = fspool.tile([P, NC, C], BF16, tag="ylem",
                                   name="ylem")
                nc.scalar.copy(ylem[:], ylb_t[:, c0:c0 + NC].unsqueeze(2)
                               .broadcast_to([P, NC, C]))
                syp = fspool.tile([P, NC, C], BF16, tag="syp", name="syp")
                nc.vector.tensor_tensor(syp[:], cie, ylem[:], ALU.is_equal)
                nc.vector.tensor_mul(syp[:], syp[:], scb[j][:])
                tree_last(syp, syp, C, ALU.add)
                nc.scalar.copy(sy_t[:, c0:c0 + NC], syp[:, :, 0:1].squeeze(2))

            # ---------- pipelined main loop ----------
            l0_t = ppool.tile([P, NPP], F32, tag="l0", name="l0")
            l1_t = ppool.tile([P, NPP], F32, tag="l1", name="l1")

            def bce():
                nc.scalar.activation(l0_t[:], obj_t[:], ACTF.Ln, bias=1.0,
                                     scale=-1.0)
                nc.scalar.activation(l1_t[:], obj_t[:], ACTF.Ln)
                nc.vector.tensor_single_scalar(l1_t[:], l1_t[:], -100.0,
                                               ALU.max)
                nc.vector.tensor_reduce(part_t[:, 1:2], l0_t[:], AX.X,
                                        ALU.add)
                nc.vector.tensor_sub(l1_t[:], l1_t[:], l0_t[:])  # logit diff

            for i in range(NCH):
                pair(i)
                if i >= 1:
                    rank(i - 1)
                if i == 1:
                    bce()
                if i >= 2:
                    phaseB(i - 2)
            rank(NCH - 1)
            phaseB(NCH - 2)
            phaseB(NCH - 1)

            # ---------- pos mask + masked sums ----------
            nc.vector.tensor_single_scalar(posf_t[:], mxf_t[:], LN_THIRD,
                                           ALU.is_ge)
            nc.vector.tensor_reduce(part_t[:, 0:1], posf_t[:], AX.X, ALU.add)
            nc.vector.tensor_mul(l1_t[:], l1_t[:], posf_t[:, 0:NPP])
            nc.vector.tensor_reduce(part_t[:, 2:3], l1_t[:], AX.X, ALU.add)

            # ---------- focal correction planes ----------
            py_t = plane("py")
            nc.scalar.activation(py_t[:], sy_t[:], ACTF.Sigmoid)
            lnpy_t = plane("lnpy")
            nc.scalar.activation(lnpy_t[:], py_t[:], ACTF.Ln)      # = -spny
            ly_t = plane("ly")
            nc.scalar.activation(ly_t[:], py_t[:], ACTF.Ln, bias=1.0,
                                 scale=-1.0)                       # ln(1-py)
            qy_t = plane("qy")
            nc.vector.tensor_scalar(qy_t[:], py_t[:], -1.0, 1.0, ALU.mult,
                                    ALU.add)
            nc.vector.tensor_mul(qy_t[:], qy_t[:], qy_t[:])
            g1_t = plane("g1")
            nc.vector.scalar_tensor_tensor(g1_t[:], lnpy_t[:], -0.25, qy_t[:],
                                           ALU.mult, ALU.mult)     # g1y
            py2_t = plane("py2")
            nc.vector.tensor_mul(py2_t[:], py_t[:], py_t[:])
            g0_t = plane("g0")
            nc.vector.scalar_tensor_tensor(g0_t[:], py2_t[:], -0.75, ly_t[:],
                                           ALU.mult, ALU.mult)     # g0y
            nc.vector.tensor_sub(g1_t[:], g1_t[:], g0_t[:])        # corr
            row_t = plane("row")
            nc.vector.scalar_tensor_tensor(row_t[:], rs0_t[:], -0.75, g1_t[:],
                                           ALU.mult, ALU.add)
            nc.vector.tensor_mul(row_t[:], row_t[:], posf_t[:])
            nc.vector.tensor_reduce(part_t[:, 4:5], row_t[:], AX.X, ALU.add)

            # ---------- GIoU planes (f32) ----------
            gme_t = plane("gme")
            nc.scalar.activation(gme_t[:], mxf_t[:], ACTF.Exp)     # g_m
            thxM = payl_t[:, 0, :]
            tlxM = payl_t[:, 1, :]
            thyM = payl_t[:, 2, :]
            tlyM = payl_t[:, 3, :]
            taeM = payl_t[:, 4, :]
            sM_t = plane("sM")
            nc.vector.tensor_tensor(sM_t[:], af_t[:, 4, :], taeM, ALU.add)
            ipM_t = plane("ipM")
            nc.vector.tensor_mul(ipM_t[:], gme_t[:], sM_t[:])
            un_t = plane("un")
            nc.vector.tensor_sub(un_t[:], sM_t[:], ipM_t[:])   # union + 1e-6
            ru_t = plane("ru")
            nc.vector.reciprocal_approx_fast(ru_t[:], un_t[:])
            iouM_t = plane("iouM")
            nc.vector.tensor_mul(iouM_t[:], ipM_t[:], ru_t[:])
            ex_t = plane("ex")
            sc2_t = plane("sc2")
            nc.vector.tensor_tensor(ex_t[:], af_t[:, 0, :], thxM, ALU.max)
            nc.vector.tensor_tensor(sc2_t[:], af_t[:, 1, :], tlxM, ALU.min)
            nc.vector.tensor_sub(ex_t[:], ex_t[:], sc2_t[:])
            ey_t = plane("ey")
            nc.vector.tensor_tensor(ey_t[:], af_t[:, 2, :], thyM, ALU.max)
            nc.vector.tensor_tensor(sc2_t[:], af_t[:, 3, :], tlyM, ALU.min)
            nc.vector.tensor_sub(ey_t[:], ey_t[:], sc2_t[:])
            nc.vector.tensor_mul(ex_t[:], ex_t[:], ey_t[:])    # enclose
            nc.vector.tensor_scalar_add(ex_t[:], ex_t[:], 1e-6)
            nc.vector.tensor_sub(ey_t[:], ex_t[:], un_t[:])    # encl - union
            nc.vector.reciprocal_approx_fast(ex_t[:], ex_t[:])
            nc.vector.tensor_mul(ey_t[:], ey_t[:], ex_t[:])
            nc.vector.tensor_sub(iouM_t[:], iouM_t[:], ey_t[:])  # giou
            nc.vector.tensor_mul(iouM_t[:], iouM_t[:], posf_t[:])
            nc.vector.tensor_scalar(iouM_t[:], iouM_t[:], -1.0, 0.0,
                                    ALU.mult, ALU.add)
            nc.vector.tensor_reduce(part_t[:, 3:4], iouM_t[:], AX.X, ALU.add)

            # ---------- cross-partition reduce + final scalars ----------
            red_t = ppool.tile([P, 8], F32)
            nc.gpsimd.partition_all_reduce(red_t[:], part_t[:], P,
                                           bass_isa.ReduceOp.add)
            r0 = red_t[0:1, :]
            out_t = ppool.tile([1, 8], F32)
            nc.vector.memset(out_t[:], 0.0)
            s1 = ppool.tile([1, 1], F32, tag="s1", name="s1")
            nc.vector.tensor_add(s1[:], r0[:, 1:2], r0[:, 2:3])
            c96 = ppool.tile([1, 1], F32, tag="c96", name="c96")
            nc.vector.memset(c96[:], float(N) * 0.5)
            s2 = ppool.tile([1, 1], F32, tag="s2", name="s2")
            nc.vector.scalar_tensor_tensor(s2[:], r0[:, 0:1], 0.5, c96[:],
                                           ALU.mult, ALU.add)
            nc.vector.scalar_tensor_tensor(out_t[:, 0:1], s1[:], -1.0, s2[:],
                                           ALU.mult, ALU.mult)
            nc.vector.tensor_add(out_t[:, 1:2], r0[:, 0:1], r0[:, 3:4])
            s3 = ppool.tile([1, 1], F32, tag="s3", name="s3")
            nc.vector.tensor_scalar(s3[:], r0[:, 0:1], float(C), 1.0,
                                    ALU.mult, ALU.max)
            nc.vector.reciprocal(s3[:], s3[:])
            nc.vector.tensor_mul(out_t[:, 2:3], r0[:, 4:5], s3[:])
            nc.vector.tensor_copy(out_t[:, 3:4], r0[:, 0:1])
            nc.sync.dma_start(out_d[:], out_t[:])

    nc.compile()
    return nc


def prep_core_inputs(objectness, boxes, class_scores, target_boxes,
                     target_labels):
    """Split full inputs into 8 per-core input maps (host-side precompute)."""
    import ml_dtypes
    bf16 = ml_dtypes.bfloat16
    objf = np.ascontiguousarray(objectness, dtype=np.float32).reshape(B, N)
    boxf = np.ascontiguousarray(boxes, dtype=np.float32).reshape(B, N, 4)
    clsf = np.ascontiguousarray(class_scores, dtype=np.float32).reshape(B, N, C)
    tbs = np.asarray(target_boxes, dtype=np.float32)
    tls = np.asarray(target_labels)

    rife1 = np.repeat((199.0 - np.arange(NT, dtype=np.float32))[:, None],
                      NC, axis=1).reshape(NT * NC)
    rife = np.broadcast_to(rife1[None, :], (P, NT * NC)).astype(bf16)
    rife21 = np.repeat((199.0 - np.arange(NT, dtype=np.float32))[None, :],
                       NC, axis=0).reshape(NC * NT)
    rife2 = np.broadcast_to(rife21[None, :], (P, NC * NT)).astype(bf16)
    cif = np.broadcast_to(np.arange(C, dtype=np.float32)[None, :],
                          (P, C)).astype(bf16)
    ident = np.eye(P, dtype=np.float32).astype(bf16)

    in_maps = []
    for b in range(B):
        obj = objf[b].reshape(P, NPP)
        bx = boxf[b].reshape(P, NPP, 4)
        cx, cy, w, h = bx[..., 0], bx[..., 1], bx[..., 2], bx[..., 3]
        af = np.zeros((P, 5, NPA), dtype=np.float32)
        af[:, 0, :NPP] = cx + 0.5 * w
        af[:, 1, :NPP] = cx - 0.5 * w
        af[:, 2, :NPP] = cy + 0.5 * h
        af[:, 3, :NPP] = cy - 0.5 * h
        af[:, 4, :NPP] = w * h
        af[:, 0, NPP:] = -20.0   # dummy anchors: zero intersection
        af[:, 1, NPP:] = -19.0
        af[:, 2, NPP:] = -20.0
        af[:, 3, NPP:] = -19.0
        af[:, 4, NPP:] = 1.0
        ab = af.astype(bf16)
        cls = np.zeros((P, NPA, C), dtype=np.float32)
        cls[:, :NPP, :] = clsf[b].reshape(P, NPP, C)
        cls = cls.reshape(P, NPA * C)
        tb = tbs[b]
        pay = np.stack([tb[:, 0] + 0.5 * tb[:, 2],
                        tb[:, 0] - 0.5 * tb[:, 2],
                        tb[:, 1] + 0.5 * tb[:, 3],
                        tb[:, 1] - 0.5 * tb[:, 3],
                        tb[:, 2] * tb[:, 3] + 1e-6,
                        tls[b].astype(np.float32)], axis=1)  # [NT, 6]
        # teall: slot-major [5, NT, NC] (payload value broadcast along NC)
        te1 = np.repeat(pay[:, 0:5].T[:, :, None], NC, axis=2)  # [5, NT, NC]
        teall = np.broadcast_to(te1.reshape(1, 5, NT * NC),
                                (P, 5, NT * NC)).astype(bf16)
        tab = np.zeros((P, 8), dtype=np.float32)
        tab[:NT, :NPAY] = pay
        in_maps.append({"obj": obj, "af": af, "ab": np.ascontiguousarray(ab),
                        "cls": cls,
                        "teall": np.ascontiguousarray(teall),
                        "rife": np.ascontiguousarray(rife),
                        "rife2": np.ascontiguousarray(rife2),
                        "tab": tab.astype(bf16),
                        "cif": np.ascontiguousarray(cif),
                        "ident": np.ascontiguousarray(ident)})
    return in_maps


def combine_outputs(outs):
    """outs: list of 8 per-core [1,8] arrays -> scalar loss."""
    o = np.stack([np.asarray(x).reshape(8) for x in outs])  # [8, 8]
    obj_terms, bb_sums, cl_sums, pcs = o[:, 0], o[:, 1], o[:, 2], o[:, 3]
    num_pos = max(float(pcs.sum()), 1.0)
    loss = (np.float32(obj_terms.sum()) / np.float32(B)
            + np.float32(5.0) * np.float32(bb_sums.sum()) / np.float32(num_pos)
            + np.float32(cl_sums.sum()) / np.float32(B))
    return np.float32(loss)


_NC_CACHE = {}


def kernel(objectness, boxes, class_scores, target_boxes, target_labels):
    from concourse.bass_utils import run_bass_kernel_spmd
    if "nc" not in _NC_CACHE:
        _NC_CACHE["nc"] = build_kernel()
    nc = _NC_CACHE["nc"]
    in_maps = prep_core_inputs(objectness, boxes, class_scores,
                               target_boxes, target_labels)
    res = run_bass_kernel_spmd(nc, in_maps, core_ids=list(range(B)))
    outs = [res.results[b]["out"] for b in range(B)]
    return combine_outputs(outs)
